# revision 20
# baseline (speedup 1.0000x reference)
"""CSUN reward kernel (retrieval_knn) on 8 Trainium2 NeuronCores.

scores[i] = stability(eah[i]) * uni[i] * nov[i]
  d_elmd(i,j) = sum_d |cumsum(a)_id - cumsum(b)_jd|   (L1 on cumsum transform)
  d_amd(i,j)  = max_d |a_id - b_jd|                   (Chebyshev)
  combine(e,c) = Ce*e/(1+e) + Ca*c/(1+c) = (Ce+Ca) - Ce/(1+e) - Ca/(1+c)
  uni[i] = sum_j combine(gen,gen) / (N-1)   nov[i] = min_j combine(gen,ref)

Sharding: ref set M=4096 split 512/core (nov), gen axis split 32/core (uni).
Per core, everything lands in [128 j-partitions, i-free] layout:
  - ELMD: ScalarE activation Abs(Bcs - Acs_i) per i -> [103d, j] abs tile ->
    PE matmul (abs tile as weights, ones as rhs) -> column i of psum [128j, i].
  - AMD: custom DVE op, running Chebyshev max over d on [128j, i-free]:
    acc = max(acc, |arep_d - ra_d|) in one instruction per d.
Combine + min-tree over chunks, PE transpose, free-axis min -> nov partials.
AllGather + strided-reduce combines partials across cores.
"""

import sys

if "/opt/trn_rl_repo" not in sys.path:
    sys.path.insert(0, "/opt/trn_rl_repo")

import numpy as np

N = 256          # gen structures
M = 4096         # ref structures
DE = 103         # elmd dim
DA = 100         # amd dim
NC = 8           # cores
MS = M // NC     # 512 ref per core
US = N // NC     # 32 gen per core (uni shard)
NJT = MS // 128  # 4 ref j-tiles per core

COEF_ELMD = float.fromhex("0x1.8d7d565a99f87p-1")
COEF_AMD = float.fromhex("0x1.ca0aa695981e5p-3")
EAH_SCALE = 0.4289

_CACHE = {}


def _register_dve_op(name, spec, subdim=False, perf_en=None):
    from concourse import dve_ops
    from concourse.dve_spec import lower, _has_src1
    from concourse.dve_uop import DveOpSpec

    for o in dve_ops.OPS:
        if o.name == name:
            return o
    row = max(dve_ops._SUB_OPCODE_FOR_NAME.values()) + 1
    assert row < 0x20
    dve_ops._SUB_OPCODE_FOR_NAME[name] = row
    shas = {}
    for ver in ("v3", "v4"):
        tmp = DveOpSpec(
            name=name, opcode=row, uops=lower(spec, ver=ver), rd1_en=_has_src1(spec)
        )
        shas[ver] = tmp.sha(ver)
    op = dve_ops.DveOp(name, spec, subdim=subdim, uops_sha=shas, perf_en=perf_en or {})
    dve_ops.OPS.append(op)
    dve_ops.CUSTOM_DVE_SPECS[name] = spec
    return op


def _get_ops():
    from concourse.dve_spec import Spec, Src0, Src1, C0, maxx

    absdiff = _register_dve_op(
        "ABSDIFF_ANT",
        Spec(
            body=maxx(Src0 - C0, C0 - Src0),
            reference=lambda in0, in1, s0, s1, imm2: np.maximum(
                in0.astype(np.float32) - s0, s0 - in0.astype(np.float32)
            ).astype(np.float32),
        ),
    )
    chebacc = _register_dve_op(
        "CHEBACC_ANT",
        Spec(
            body=maxx(maxx(Src0 - C0, C0 - Src0), Src1),
            reference=lambda in0, in1, s0, s1, imm2: np.maximum(
                np.maximum(in0.astype(np.float32) - s0, s0 - in0.astype(np.float32)),
                in1.astype(np.float32),
            ).astype(np.float32),
        ),
    )
    return absdiff, chebacc


def _build():
    from concourse import bacc, tile, mybir

    f32 = mybir.dt.float32
    bf16 = mybir.dt.float16  # fp16: same DVE perf modes, 8x finer ulp here
    Alu = mybir.AluOpType
    AFT = mybir.ActivationFunctionType
    X = mybir.AxisListType.X

    ABSDIFF, CHEBACC = _get_ops()

    nc = bacc.Bacc("TRN2", target_bir_lowering=False, debug=False, num_devices=NC)

    ge = nc.dram_tensor("ge", [N, DE], f32, kind="ExternalInput").ap()
    ga = nc.dram_tensor("ga", [N, DA], f32, kind="ExternalInput").ap()
    re = nc.dram_tensor("re", [MS, DE], f32, kind="ExternalInput").ap()
    ra = nc.dram_tensor("ra", [MS, DA], f32, kind="ExternalInput").ap()
    mge = nc.dram_tensor("mge", [US, DE], f32, kind="ExternalInput").ap()
    mga = nc.dram_tensor("mga", [US, DA], f32, kind="ExternalInput").ap()
    eah = nc.dram_tensor("eah", [1, N], f32, kind="ExternalInput").ap()
    tri = nc.dram_tensor("tri", [DE, DE], f32, kind="ExternalInput").ap()
    eye = nc.dram_tensor("eye", [128, 128], f32, kind="ExternalInput").ap()
    out = nc.dram_tensor("scores", [1, N], f32, kind="ExternalOutput").ap()
    dbg_nov = nc.dram_tensor("dbg_nov", [1, N], f32, kind="ExternalOutput").ap()
    dbg_uni = nc.dram_tensor("dbg_uni", [1, N], f32, kind="ExternalOutput").ap()
    dbg_stab = nc.dram_tensor("dbg_stab", [1, N], f32, kind="ExternalOutput").ap()

    CE, CA = COEF_ELMD, COEF_AMD

    with tile.TileContext(nc) as tc:
        with (
            tc.tile_pool(name="const", bufs=1) as cpool,
            tc.tile_pool(name="work", bufs=1) as wpool,
            tc.tile_pool(name="abs", bufs=4) as apool,
            tc.tile_pool(name="acc", bufs=2) as accpool,
            tc.tile_pool(name="comb", bufs=1) as combpool,
            tc.tile_pool(name="psA", bufs=4, space="PSUM") as psA,
            tc.tile_pool(name="psB", bufs=2, space="PSUM") as psB,
            tc.tile_pool(name="psC", bufs=2, space="PSUM") as psC,
            tc.tile_pool(name="dram", bufs=1, space="DRAM") as dpool,
        ):
            # ---------------- prep ----------------
            tri_s = cpool.tile([DE, DE], f32)
            nc.sync.dma_start(tri_s[:], tri[:])
            eye_s = cpool.tile([128, 128], f32)
            nc.sync.dma_start(eye_s[:], eye[:])

            # per-partition scalar sources (also reused as transpose sources)
            ra_f = []
            for jt in range(NJT):
                t_f = cpool.tile([128, DA], f32, tag=f"raf{jt}", name=f"raf{jt}")
                nc.sync.dma_start(t_f[:], ra[jt * 128 : (jt + 1) * 128, :])
                ra_f.append(t_f)
            ga_s = []
            for ch in range(2):
                t_f = cpool.tile([128, DA], f32, tag=f"gas{ch}", name=f"gas{ch}")
                nc.sync.dma_start(t_f[:], ga[ch * 128 : (ch + 1) * 128, :])
                ga_s.append(t_f)

            # raw row-major loads used only as PE-transpose sources
            ge_r = []
            for ch in range(2):
                t = wpool.tile([128, DE], f32, tag=f"ger{ch}", name=f"ger{ch}")
                nc.sync.dma_start(t[:], ge[ch * 128 : (ch + 1) * 128, :])
                ge_r.append(t)
            re_r = []
            for jt in range(NJT):
                t = wpool.tile([128, DE], f32, tag=f"rer{jt}", name=f"rer{jt}")
                nc.sync.dma_start(t[:], re[jt * 128 : (jt + 1) * 128, :])
                re_r.append(t)
            mge_r = wpool.tile([US, DE], f32)
            nc.sync.dma_start(mge_r[:], mge[:])
            mga_r = wpool.tile([US, DA], f32)
            nc.sync.dma_start(mga_r[:], mga[:])

            # transpose via PE (x.T @ I), evacuate PSUM on ScalarE
            geT = wpool.tile([DE, N], f32)
            reT = wpool.tile([DE, MS], f32)
            gaT = wpool.tile([DA, N], f32)
            mgeT = wpool.tile([DE, US], f32)
            mgaT = wpool.tile([DA, US], f32)
            prep_tr = [
                (ge_r[0], geT[:, 0:128]),
                (ge_r[1], geT[:, 128:256]),
                (re_r[0], reT[:, 0:128]),
                (re_r[1], reT[:, 128:256]),
                (re_r[2], reT[:, 256:384]),
                (re_r[3], reT[:, 384:512]),
                (ga_s[0], gaT[:, 0:128]),
                (ga_s[1], gaT[:, 128:256]),
            ]
            for k, (src, dst) in enumerate(prep_tr):
                dcols = src.shape[1]
                tr = psB.tile([dcols, 128], f32, tag="tr", name=f"ptr{k}")
                nc.tensor.transpose(tr[:], src[:], eye_s[:])
                nc.scalar.copy(out=dst, in_=tr[:])
            tr_m = psB.tile([DE, US], f32, tag="tr", name="ptr_mge")
            nc.tensor.transpose(tr_m[:], mge_r[:], eye_s[0:US, 0:US])
            nc.scalar.copy(out=mgeT[:], in_=tr_m[:])
            tr_g = psB.tile([DA, US], f32, tag="tr", name="ptr_mga")
            nc.tensor.transpose(tr_g[:], mga_r[:], eye_s[0:US, 0:US])
            nc.scalar.copy(out=mgaT[:], in_=tr_g[:])

            # cumsum along d via upper-tri matmul: cs^T[d,i] = sum_{e<=d} x^T[e,i]
            csA_ps = psA.tile([DE, N], f32, tag="big")
            nc.tensor.matmul(csA_ps[:], tri_s[:], geT[:], start=True, stop=True)
            csB_ps = psA.tile([DE, MS], f32, tag="big")
            nc.tensor.matmul(csB_ps[:], tri_s[:], reT[:], start=True, stop=True)
            csM_ps = psA.tile([DE, US], f32, tag="big")
            nc.tensor.matmul(csM_ps[:], tri_s[:], mgeT[:], start=True, stop=True)

            Acs = cpool.tile([DE, N], bf16)
            nc.vector.tensor_copy(Acs[:], csA_ps[:])
            AcsN = cpool.tile([DE, N], bf16)  # negated fp16 cumsum, ACT bias
            nc.vector.tensor_scalar(
                out=AcsN[:], in0=Acs[:], scalar1=-1.0, scalar2=None, op0=Alu.mult
            )
            Bcs = cpool.tile([DE, MS], bf16)
            nc.vector.tensor_copy(Bcs[:], csB_ps[:])
            # round through bf16 so |Acs_i - Mcs_j| is exactly 0 on the uni
            # diagonal (saturating e/(1+e) otherwise turns rounding noise
            # into a large spurious diagonal term)
            McsH = cpool.tile([DE, US], bf16)
            nc.vector.tensor_copy(McsH[:], csM_ps[:])
            McsN = cpool.tile([DE, US], bf16)
            nc.vector.tensor_scalar(
                out=McsN[:], in0=McsH[:], scalar1=-1.0, scalar2=None, op0=Alu.mult
            )

            # replicated gen_amd^T, d-major: arep[p, d*N+i] = ga[i, d]
            gaT_bf = wpool.tile([DA, N], bf16)
            nc.vector.tensor_copy(gaT_bf[:], gaT[:])
            flat_d = dpool.tile([1, DA * N], bf16)
            nc.sync.dma_start(flat_d[:], gaT_bf[:])
            arep = cpool.tile([128, DA * N], bf16)
            DCH = 20  # d's per chunk: replicate via 0-stride DMA reads
            for c0 in range(0, DA, DCH):
                nc.sync.dma_start(
                    arep[:, c0 * N : (c0 + DCH) * N],
                    flat_d[:, c0 * N : (c0 + DCH) * N].partition_broadcast(128),
                )

            # replicated my-gen amd^T for uni: mrep[p, d*US+j] = mga[j, d]
            mgaT_bf = wpool.tile([DA, US], bf16)
            nc.vector.tensor_copy(mgaT_bf[:], mgaT[:])
            mflat_d = dpool.tile([1, DA * US], bf16)
            nc.sync.dma_start(mflat_d[:], mgaT_bf[:])
            mrep = cpool.tile([128, DA * US], bf16)
            nc.sync.dma_start(mrep[:], mflat_d.partition_broadcast(128))

            eah_s = cpool.tile([1, N], f32)
            nc.sync.dma_start(eah_s[:], eah[:])

            ones_e = cpool.tile([DE, 1], bf16)
            nc.vector.memset(ones_e[:], 1.0)

            # ---- ELMD nov via min identity: |a-b| = a+b-2*min(a,b) ------
            # ET[jt][j, i] = sum_d min(A_id, B_jd); SAB[jt][j, i] = 1+SA_i+SB_j
            # row/col sums SA [1,N], SB [1,MS] via PE ones-matmuls
            ET_ps = [
                psA.tile([128, N], f32, tag="big", name=f"ET{k}") for k in range(NJT)
            ]
            for i in range(N):
                ab = apool.tile([DE, MS], bf16, tag="ab")
                nc.scalar.activation(
                    out=ab[:],
                    in_=Bcs[:],
                    func=AFT.Abs,
                    bias=AcsN[:, i : i + 1],
                    scale=1.0,
                )
                for jt in range(NJT):
                    nc.tensor.matmul(
                        ET_ps[jt][:, i : i + 1],
                        ab[:, jt * 128 : (jt + 1) * 128],
                        ones_e[:],
                        start=True,
                        stop=True,
                    )

            # ---- AMD nov + combine per j-tile -> comb[jt] [128j, 256i] ----
            accs = [
                accpool.tile([128, N], bf16, tag=f"acc{jt}", name=f"acc{jt}")
                for jt in range(NJT)
            ]
            for jt in range(NJT):
                nc.vector._custom_dve(
                    ABSDIFF,
                    out=accs[jt][:],
                    in0=arep[:, 0:N],
                    s0=ra_f[jt][:, 0:1],
                )
            for d in range(1, DA):
                for jt in range(NJT):
                    nc.vector._custom_dve(
                        CHEBACC,
                        out=accs[jt][:],
                        in0=arep[:, d * N : (d + 1) * N],
                        in1=accs[jt][:],
                        s0=ra_f[jt][:, d : d + 1],
                    )
            comb = []
            for jt in range(NJT):
                # affine stages on ScalarE (ACT), reciprocals on DVE
                c1f = accpool.tile([128, N], f32, tag="c1f")
                nc.scalar.activation(
                    out=c1f[:], in_=accs[jt][:], func=AFT.Copy, bias=1.0, scale=1.0
                )
                r_a = accpool.tile([128, N], f32, tag="r_a")
                rsc = accpool.tile([128, N], f32, tag="rsc")
                nc.vector.reciprocal_approx_accurate(out=r_a[:], in_=c1f[:], scratch=rsc[:])
                # x = (CE+CA) - CA*r_a
                x = accpool.tile([128, N], f32, tag="x")
                nc.scalar.activation(
                    out=x[:], in_=r_a[:], func=AFT.Copy, bias=CE + CA, scale=-CA
                )
                e1 = accpool.tile([128, N], f32, tag="e1")
                nc.scalar.activation(
                    out=e1[:], in_=ET_ps[jt][:], func=AFT.Copy, bias=1.0, scale=1.0
                )
                r_e = accpool.tile([128, N], f32, tag="r_e")
                rsc2 = accpool.tile([128, N], f32, tag="rsc2")
                nc.vector.reciprocal_approx_accurate(out=r_e[:], in_=e1[:], scratch=rsc2[:])
                cb = combpool.tile([128, N], f32, tag=f"comb{jt}", name=f"comb{jt}")
                nc.vector.scalar_tensor_tensor(
                    out=cb[:],
                    in0=r_e[:],
                    scalar=-CE,
                    in1=x[:],
                    op0=Alu.mult,
                    op1=Alu.add,
                )
                comb.append(cb)

            # min over the 4 chunks, then transpose + free-axis min over j
            nc.vector.tensor_tensor(comb[0][:], comb[0][:], comb[1][:], Alu.min)
            nc.vector.tensor_tensor(comb[2][:], comb[2][:], comb[3][:], Alu.min)
            nc.vector.tensor_tensor(comb[0][:], comb[0][:], comb[2][:], Alu.min)
            nov_p = []
            for h in range(2):
                tr = psB.tile([128, 128], f32, tag="tr", name=f"tr{h}")
                nc.tensor.transpose(tr[:], comb[0][:, h * 128 : (h + 1) * 128], eye_s[:])
                np_t = combpool.tile([128, 1], f32, tag=f"novp{h}", name=f"novp{h}")
                nc.vector.tensor_reduce(out=np_t[:], in_=tr[:], axis=X, op=Alu.min)
                nov_p.append(np_t)

            # ---- uni (gen x gen, my 32 j columns): [128i, 32j] layout -----
            uET_ps = [psC.tile([128, US], f32, tag="u", name=f"uET{k}") for k in range(2)]
            for j in range(US):
                ab = apool.tile([DE, N], bf16, tag="uab")
                nc.scalar.activation(
                    out=ab[:],
                    in_=Acs[:],
                    func=AFT.Abs,
                    bias=McsN[:, j : j + 1],
                    scale=1.0,
                )
                for ch in range(2):
                    nc.tensor.matmul(
                        uET_ps[ch][:, j : j + 1],
                        ab[:, ch * 128 : (ch + 1) * 128],
                        ones_e[:],
                        start=True,
                        stop=True,
                    )
            uni_p = []
            for ch in range(2):
                uacc = wpool.tile([128, US], bf16, tag=f"uacc{ch}", name=f"uacc{ch}")
                nc.vector._custom_dve(
                    ABSDIFF,
                    out=uacc[:],
                    in0=mrep[:, 0:US],
                    s0=ga_s[ch][:, 0:1],
                )
                for d in range(1, DA):
                    nc.vector._custom_dve(
                        CHEBACC,
                        out=uacc[:],
                        in0=mrep[:, d * US : (d + 1) * US],
                        in1=uacc[:],
                        s0=ga_s[ch][:, d : d + 1],
                    )
                uc1 = wpool.tile([128, US], f32, tag=f"uc1{ch}", name=f"uc1{ch}")
                nc.vector.tensor_scalar(
                    out=uc1[:], in0=uacc[:], scalar1=1.0, scalar2=None, op0=Alu.add
                )
                ur_a = wpool.tile([128, US], f32, tag=f"ura{ch}", name=f"ura{ch}")
                ursc = wpool.tile([128, US], f32, tag=f"ursc{ch}", name=f"ursc{ch}")
                nc.vector.reciprocal_approx_accurate(out=ur_a[:], in_=uc1[:], scratch=ursc[:])
                ux = wpool.tile([128, US], f32, tag=f"ux{ch}", name=f"ux{ch}")
                nc.vector.tensor_scalar(
                    out=ux[:],
                    in0=ur_a[:],
                    scalar1=-CA,
                    scalar2=CE + CA,
                    op0=Alu.mult,
                    op1=Alu.add,
                )
                ue1 = wpool.tile([128, US], f32, tag=f"ue1{ch}", name=f"ue1{ch}")
                nc.scalar.activation(
                    out=ue1[:], in_=uET_ps[ch][:], func=AFT.Copy, bias=1.0, scale=1.0
                )
                ur_e = wpool.tile([128, US], f32, tag=f"ure{ch}", name=f"ure{ch}")
                ursc2 = wpool.tile([128, US], f32, tag=f"ursc2{ch}", name=f"ursc2{ch}")
                nc.vector.reciprocal_approx_accurate(out=ur_e[:], in_=ue1[:], scratch=ursc2[:])
                ucomb = wpool.tile([128, US], f32, tag=f"ucomb{ch}", name=f"ucomb{ch}")
                nc.vector.scalar_tensor_tensor(
                    out=ucomb[:],
                    in0=ur_e[:],
                    scalar=-CE,
                    in1=ux[:],
                    op0=Alu.mult,
                    op1=Alu.add,
                )
                up = wpool.tile([128, 1], f32, tag=f"up{ch}", name=f"up{ch}")
                nc.vector.tensor_reduce(out=up[:], in_=ucomb[:], axis=X, op=Alu.add)
                uni_p.append(up)

            # ---------------- allgather partials ---------------------------
            cin = dpool.tile([1, 4 * 128], f32)
            nc.sync.dma_start(cin[:, 0:128], nov_p[0][:])
            nc.sync.dma_start(cin[:, 128:256], nov_p[1][:])
            nc.sync.dma_start(cin[:, 256:384], uni_p[0][:])
            nc.sync.dma_start(cin[:, 384:512], uni_p[1][:])
            cout = dpool.tile([NC, 4 * 128], f32)
            nc.gpsimd.collective_compute(
                "AllGather",
                Alu.bypass,
                replica_groups=[list(range(NC))],
                ins=[cin.opt()],
                outs=[cout.opt()],
            )
            nov_cat = wpool.tile([1, NC * N], f32)
            nc.sync.dma_start(nov_cat[:], cout[:, 0:N])
            uni_cat = wpool.tile([1, NC * N], f32)
            nc.sync.dma_start(uni_cat[:], cout[:, N : 2 * N])
            # reduce over the 8 core-chunks: view [1, (c n)] as [1, n, c]
            nov_r = wpool.tile([1, N], f32)
            nc.vector.tensor_reduce(
                out=nov_r[:],
                in_=nov_cat.rearrange("p (c n) -> p n c", c=NC),
                axis=X,
                op=Alu.min,
            )
            uni_r = wpool.tile([1, N], f32)
            nc.vector.tensor_reduce(
                out=uni_r[:],
                in_=uni_cat.rearrange("p (c n) -> p n c", c=NC),
                axis=X,
                op=Alu.add,
            )

            # ---------------- final ----------------------------------------
            stab = wpool.tile([1, N], f32)
            nc.vector.tensor_scalar(
                out=stab[:],
                in0=eah_s[:],
                scalar1=-1.0 / EAH_SCALE,
                scalar2=1.0,
                op0=Alu.mult,
                op1=Alu.add,
            )
            nc.vector.tensor_scalar(
                out=stab[:],
                in0=stab[:],
                scalar1=0.0,
                scalar2=1.0,
                op0=Alu.max,
                op1=Alu.min,
            )
            s1 = wpool.tile([1, N], f32)
            nc.vector.tensor_tensor(s1[:], stab[:], nov_r[:], Alu.mult)
            s2 = wpool.tile([1, N], f32)
            nc.vector.tensor_scalar(
                out=s2[:],
                in0=uni_r[:],
                scalar1=1.0 / (N - 1),
                scalar2=None,
                op0=Alu.mult,
            )
            sc = wpool.tile([1, N], f32)
            nc.vector.tensor_tensor(sc[:], s1[:], s2[:], Alu.mult)
            nc.sync.dma_start(out[:], sc[:])
            nc.sync.dma_start(dbg_nov[:], nov_r[:])
            nc.sync.dma_start(dbg_uni[:], uni_r[:])
            nc.sync.dma_start(dbg_stab[:], stab[:])

    nc.compile()
    return nc


def _get_nc():
    if "nc" not in _CACHE:
        _CACHE["nc"] = _build()
    return _CACHE["nc"]


def _in_maps(inputs):
    ge = np.ascontiguousarray(inputs["gen_embs_elmd"], dtype=np.float32)
    ga = np.ascontiguousarray(inputs["gen_embs_amd"], dtype=np.float32)
    re = np.ascontiguousarray(inputs["ref_embs_elmd"], dtype=np.float32)
    ra = np.ascontiguousarray(inputs["ref_embs_amd"], dtype=np.float32)
    eah = np.ascontiguousarray(inputs["e_above_hull"], dtype=np.float32).reshape(1, N)
    tri = np.triu(np.ones((DE, DE), dtype=np.float32))
    eye = np.eye(128, dtype=np.float32)
    maps = []
    for c in range(NC):
        maps.append(
            {
                "ge": ge,
                "ga": ga,
                "re": np.ascontiguousarray(re[c * MS : (c + 1) * MS]),
                "ra": np.ascontiguousarray(ra[c * MS : (c + 1) * MS]),
                "mge": np.ascontiguousarray(ge[c * US : (c + 1) * US]),
                "mga": np.ascontiguousarray(ga[c * US : (c + 1) * US]),
                "eah": eah,
                "tri": tri,
                "eye": eye,
            }
        )
    return maps


def kernel(**inputs) -> np.ndarray:
    from concourse.bass_utils import run_bass_kernel_spmd

    nc = _get_nc()
    res = run_bass_kernel_spmd(nc, _in_maps(inputs), list(range(NC)))
    return np.asarray(res.results[0]["scores"], dtype=np.float32).reshape(N)


# revision 21
# speedup vs baseline: 1.1113x; 1.1113x over previous
"""CSUN reward kernel (retrieval_knn) on 8 Trainium2 NeuronCores.

scores[i] = stability(eah[i]) * uni[i] * nov[i]
  d_elmd(i,j) = sum_d |cumsum(a)_id - cumsum(b)_jd|   (L1 on cumsum transform)
  d_amd(i,j)  = max_d |a_id - b_jd|                   (Chebyshev)
  combine(e,c) = Ce*e/(1+e) + Ca*c/(1+c) = (Ce+Ca) - Ce/(1+e) - Ca/(1+c)
  uni[i] = sum_j combine(gen,gen) / (N-1)   nov[i] = min_j combine(gen,ref)

Sharding: ref set M=4096 split 512/core (nov), gen axis split 32/core (uni).
Per core, everything lands in [128 j-partitions, i-free] layout:
  - ELMD: ScalarE activation Abs(Bcs - Acs_i) per i -> [103d, j] abs tile ->
    PE matmul (abs tile as weights, ones as rhs) -> column i of psum [128j, i].
  - AMD: custom DVE op, running Chebyshev max over d on [128j, i-free]:
    acc = max(acc, |arep_d - ra_d|) in one instruction per d.
Combine + min-tree over chunks, PE transpose, free-axis min -> nov partials.
AllGather + strided-reduce combines partials across cores.
"""

import sys

if "/opt/trn_rl_repo" not in sys.path:
    sys.path.insert(0, "/opt/trn_rl_repo")

import numpy as np

N = 256          # gen structures
M = 4096         # ref structures
DE = 103         # elmd dim
DA = 100         # amd dim
NC = 8           # cores
MS = M // NC     # 512 ref per core
US = N // NC     # 32 gen per core (uni shard)
NJT = MS // 128  # 4 ref j-tiles per core

COEF_ELMD = float.fromhex("0x1.8d7d565a99f87p-1")
COEF_AMD = float.fromhex("0x1.ca0aa695981e5p-3")
EAH_SCALE = 0.4289

_CACHE = {}


def _register_dve_op(name, spec, subdim=False, perf_en=None):
    from concourse import dve_ops
    from concourse.dve_spec import lower, _has_src1
    from concourse.dve_uop import DveOpSpec

    for o in dve_ops.OPS:
        if o.name == name:
            return o
    row = max(dve_ops._SUB_OPCODE_FOR_NAME.values()) + 1
    assert row < 0x20
    dve_ops._SUB_OPCODE_FOR_NAME[name] = row
    shas = {}
    for ver in ("v3", "v4"):
        tmp = DveOpSpec(
            name=name, opcode=row, uops=lower(spec, ver=ver), rd1_en=_has_src1(spec)
        )
        shas[ver] = tmp.sha(ver)
    op = dve_ops.DveOp(name, spec, subdim=subdim, uops_sha=shas, perf_en=perf_en or {})
    dve_ops.OPS.append(op)
    dve_ops.CUSTOM_DVE_SPECS[name] = spec
    return op


def _get_ops():
    from concourse.dve_spec import Spec, Src0, Src1, C0, C1, maxx

    absdiff = _register_dve_op(
        "ABSDIFF_ANT",
        Spec(
            body=maxx(Src0 - C0, C0 - Src0),
            reference=lambda in0, in1, s0, s1, imm2: np.maximum(
                in0.astype(np.float32) - s0, s0 - in0.astype(np.float32)
            ).astype(np.float32),
        ),
    )
    pairmax = _register_dve_op(
        "PAIRMAX_ANT",
        Spec(
            body=maxx(maxx(Src0 - C0, C0 - Src0), maxx(Src1 - C1, C1 - Src1)),
            reference=lambda in0, in1, s0, s1, imm2: np.maximum(
                np.maximum(
                    in0.astype(np.float32) - s0, s0 - in0.astype(np.float32)
                ),
                np.maximum(
                    in1.astype(np.float32) - s1, s1 - in1.astype(np.float32)
                ),
            ).astype(np.float32),
        ),
    )
    chebacc = _register_dve_op(
        "CHEBACC_ANT",
        Spec(
            body=maxx(maxx(Src0 - C0, C0 - Src0), Src1),
            reference=lambda in0, in1, s0, s1, imm2: np.maximum(
                np.maximum(in0.astype(np.float32) - s0, s0 - in0.astype(np.float32)),
                in1.astype(np.float32),
            ).astype(np.float32),
        ),
    )
    return absdiff, chebacc, pairmax


def _build():
    from concourse import bacc, tile, mybir

    f32 = mybir.dt.float32
    bf16 = mybir.dt.float16  # fp16: same DVE perf modes, 8x finer ulp here
    Alu = mybir.AluOpType
    AFT = mybir.ActivationFunctionType
    X = mybir.AxisListType.X

    ABSDIFF, CHEBACC, PAIRMAX = _get_ops()

    nc = bacc.Bacc("TRN2", target_bir_lowering=False, debug=False, num_devices=NC)

    ge = nc.dram_tensor("ge", [N, DE], f32, kind="ExternalInput").ap()
    ga = nc.dram_tensor("ga", [N, DA], f32, kind="ExternalInput").ap()
    re = nc.dram_tensor("re", [MS, DE], f32, kind="ExternalInput").ap()
    ra = nc.dram_tensor("ra", [MS, DA], f32, kind="ExternalInput").ap()
    mge = nc.dram_tensor("mge", [US, DE], f32, kind="ExternalInput").ap()
    mga = nc.dram_tensor("mga", [US, DA], f32, kind="ExternalInput").ap()
    eah = nc.dram_tensor("eah", [1, N], f32, kind="ExternalInput").ap()
    tri = nc.dram_tensor("tri", [DE, DE], f32, kind="ExternalInput").ap()
    eye = nc.dram_tensor("eye", [128, 128], f32, kind="ExternalInput").ap()
    out = nc.dram_tensor("scores", [1, N], f32, kind="ExternalOutput").ap()
    dbg_nov = nc.dram_tensor("dbg_nov", [1, N], f32, kind="ExternalOutput").ap()
    dbg_uni = nc.dram_tensor("dbg_uni", [1, N], f32, kind="ExternalOutput").ap()
    dbg_stab = nc.dram_tensor("dbg_stab", [1, N], f32, kind="ExternalOutput").ap()

    CE, CA = COEF_ELMD, COEF_AMD

    with tile.TileContext(nc) as tc:
        with (
            tc.tile_pool(name="const", bufs=1) as cpool,
            tc.tile_pool(name="work", bufs=1) as wpool,
            tc.tile_pool(name="abs", bufs=4) as apool,
            tc.tile_pool(name="acc", bufs=2) as accpool,
            tc.tile_pool(name="pm", bufs=8) as pmpool,
            tc.tile_pool(name="comb", bufs=1) as combpool,
            tc.tile_pool(name="psA", bufs=4, space="PSUM") as psA,
            tc.tile_pool(name="psB", bufs=2, space="PSUM") as psB,
            tc.tile_pool(name="psC", bufs=2, space="PSUM") as psC,
            tc.tile_pool(name="dram", bufs=1, space="DRAM") as dpool,
        ):
            # ---------------- prep ----------------
            tri_s = cpool.tile([DE, DE], f32)
            nc.sync.dma_start(tri_s[:], tri[:])
            eye_s = cpool.tile([128, 128], f32)
            nc.sync.dma_start(eye_s[:], eye[:])

            # per-partition scalar sources (also reused as transpose sources)
            ra_f = []
            for jt in range(NJT):
                t_f = cpool.tile([128, DA], f32, tag=f"raf{jt}", name=f"raf{jt}")
                nc.sync.dma_start(t_f[:], ra[jt * 128 : (jt + 1) * 128, :])
                ra_f.append(t_f)
            ga_s = []
            for ch in range(2):
                t_f = cpool.tile([128, DA], f32, tag=f"gas{ch}", name=f"gas{ch}")
                nc.sync.dma_start(t_f[:], ga[ch * 128 : (ch + 1) * 128, :])
                ga_s.append(t_f)

            # raw row-major loads used only as PE-transpose sources
            ge_r = []
            for ch in range(2):
                t = wpool.tile([128, DE], f32, tag=f"ger{ch}", name=f"ger{ch}")
                nc.sync.dma_start(t[:], ge[ch * 128 : (ch + 1) * 128, :])
                ge_r.append(t)
            re_r = []
            for jt in range(NJT):
                t = wpool.tile([128, DE], f32, tag=f"rer{jt}", name=f"rer{jt}")
                nc.sync.dma_start(t[:], re[jt * 128 : (jt + 1) * 128, :])
                re_r.append(t)
            mge_r = wpool.tile([US, DE], f32)
            nc.sync.dma_start(mge_r[:], mge[:])
            mga_r = wpool.tile([US, DA], f32)
            nc.sync.dma_start(mga_r[:], mga[:])

            # transpose via PE (x.T @ I), evacuate PSUM on ScalarE
            geT = wpool.tile([DE, N], f32)
            reT = wpool.tile([DE, MS], f32)
            gaT = wpool.tile([DA, N], f32)
            mgeT = wpool.tile([DE, US], f32)
            mgaT = wpool.tile([DA, US], f32)
            prep_tr = [
                (ge_r[0], geT[:, 0:128]),
                (ge_r[1], geT[:, 128:256]),
                (re_r[0], reT[:, 0:128]),
                (re_r[1], reT[:, 128:256]),
                (re_r[2], reT[:, 256:384]),
                (re_r[3], reT[:, 384:512]),
                (ga_s[0], gaT[:, 0:128]),
                (ga_s[1], gaT[:, 128:256]),
            ]
            for k, (src, dst) in enumerate(prep_tr):
                dcols = src.shape[1]
                tr = psB.tile([dcols, 128], f32, tag="tr", name=f"ptr{k}")
                nc.tensor.transpose(tr[:], src[:], eye_s[:])
                nc.scalar.copy(out=dst, in_=tr[:])
            tr_m = psB.tile([DE, US], f32, tag="tr", name="ptr_mge")
            nc.tensor.transpose(tr_m[:], mge_r[:], eye_s[0:US, 0:US])
            nc.scalar.copy(out=mgeT[:], in_=tr_m[:])
            tr_g = psB.tile([DA, US], f32, tag="tr", name="ptr_mga")
            nc.tensor.transpose(tr_g[:], mga_r[:], eye_s[0:US, 0:US])
            nc.scalar.copy(out=mgaT[:], in_=tr_g[:])

            # cumsum along d via upper-tri matmul: cs^T[d,i] = sum_{e<=d} x^T[e,i]
            csA_ps = psA.tile([DE, N], f32, tag="big")
            nc.tensor.matmul(csA_ps[:], tri_s[:], geT[:], start=True, stop=True)
            csB_ps = psA.tile([DE, MS], f32, tag="big")
            nc.tensor.matmul(csB_ps[:], tri_s[:], reT[:], start=True, stop=True)
            csM_ps = psA.tile([DE, US], f32, tag="big")
            nc.tensor.matmul(csM_ps[:], tri_s[:], mgeT[:], start=True, stop=True)

            Acs = cpool.tile([DE, N], bf16)
            nc.vector.tensor_copy(Acs[:], csA_ps[:])
            AcsN = cpool.tile([DE, N], bf16)  # negated fp16 cumsum, ACT bias
            nc.vector.tensor_scalar(
                out=AcsN[:], in0=Acs[:], scalar1=-1.0, scalar2=None, op0=Alu.mult
            )
            Bcs = cpool.tile([DE, MS], bf16)
            nc.vector.tensor_copy(Bcs[:], csB_ps[:])
            # round through bf16 so |Acs_i - Mcs_j| is exactly 0 on the uni
            # diagonal (saturating e/(1+e) otherwise turns rounding noise
            # into a large spurious diagonal term)
            McsH = cpool.tile([DE, US], bf16)
            nc.vector.tensor_copy(McsH[:], csM_ps[:])
            McsN = cpool.tile([DE, US], bf16)
            nc.vector.tensor_scalar(
                out=McsN[:], in0=McsH[:], scalar1=-1.0, scalar2=None, op0=Alu.mult
            )

            # replicated gen_amd^T, d-major: arep[p, d*N+i] = ga[i, d]
            gaT_bf = wpool.tile([DA, N], bf16)
            nc.vector.tensor_copy(gaT_bf[:], gaT[:])
            flat_d = dpool.tile([1, DA * N], bf16)
            nc.sync.dma_start(flat_d[:], gaT_bf[:])
            arep = cpool.tile([128, DA * N], bf16)
            DCH = 20  # d's per chunk: replicate via 0-stride DMA reads
            for c0 in range(0, DA, DCH):
                nc.sync.dma_start(
                    arep[:, c0 * N : (c0 + DCH) * N],
                    flat_d[:, c0 * N : (c0 + DCH) * N].partition_broadcast(128),
                )

            # replicated my-gen amd^T for uni: mrep[p, d*US+j] = mga[j, d]
            mgaT_bf = wpool.tile([DA, US], bf16)
            nc.vector.tensor_copy(mgaT_bf[:], mgaT[:])
            mflat_d = dpool.tile([1, DA * US], bf16)
            nc.sync.dma_start(mflat_d[:], mgaT_bf[:])
            mrep = cpool.tile([128, DA * US], bf16)
            nc.sync.dma_start(mrep[:], mflat_d.partition_broadcast(128))

            eah_s = cpool.tile([1, N], f32)
            nc.sync.dma_start(eah_s[:], eah[:])

            ones_e = cpool.tile([DE, 1], bf16)
            nc.vector.memset(ones_e[:], 1.0)

            # ---- ELMD nov via min identity: |a-b| = a+b-2*min(a,b) ------
            # ET[jt][j, i] = sum_d min(A_id, B_jd); SAB[jt][j, i] = 1+SA_i+SB_j
            # row/col sums SA [1,N], SB [1,MS] via PE ones-matmuls
            ET_ps = [
                psA.tile([128, N], f32, tag="big", name=f"ET{k}") for k in range(NJT)
            ]
            for i in range(N):
                ab = apool.tile([DE, MS], bf16, tag="ab")
                nc.scalar.activation(
                    out=ab[:],
                    in_=Bcs[:],
                    func=AFT.Abs,
                    bias=AcsN[:, i : i + 1],
                    scale=1.0,
                )
                for jt in range(NJT):
                    nc.tensor.matmul(
                        ET_ps[jt][:, i : i + 1],
                        ab[:, jt * 128 : (jt + 1) * 128],
                        ones_e[:],
                        start=True,
                        stop=True,
                    )

            # ---- AMD nov + combine per j-tile -> comb[jt] [128j, 256i] ----
            def cheb_tree(pool, width, in_of, s_of, tag):
                # pairwise |a-b| max over d via PAIRMAX + streaming binary
                # merge tree of fp16 tensor_tensor(max); max ~log2 live tiles
                stack = []  # (level, tile)
                for k in range(DA // 2):
                    t = pool.tile([128, width], bf16, tag=tag, name=f"{tag}_{k}")
                    nc.vector._custom_dve(
                        PAIRMAX,
                        out=t[:],
                        in0=in_of(2 * k),
                        in1=in_of(2 * k + 1),
                        s0=s_of(2 * k),
                        s1=s_of(2 * k + 1),
                    )
                    lvl = 0
                    while stack and stack[-1][0] == lvl:
                        _, prev = stack.pop()
                        nc.vector.tensor_tensor(prev[:], prev[:], t[:], Alu.max)
                        t = prev
                        lvl += 1
                    stack.append((lvl, t))
                while len(stack) > 1:
                    _, a = stack.pop()
                    _, b = stack.pop()
                    nc.vector.tensor_tensor(b[:], b[:], a[:], Alu.max)
                    stack.append((99, b))
                return stack[0][1]

            accs = []
            for jt in range(NJT):
                accs.append(
                    cheb_tree(
                        pmpool,
                        N,
                        lambda d: arep[:, d * N : (d + 1) * N],
                        lambda d, jt=jt: ra_f[jt][:, d : d + 1],
                        f"pm{jt}",
                    )
                )
            comb = []
            for jt in range(NJT):
                # affine stages on ScalarE (ACT), reciprocals on DVE
                c1f = accpool.tile([128, N], f32, tag="c1f")
                nc.scalar.activation(
                    out=c1f[:], in_=accs[jt][:], func=AFT.Copy, bias=1.0, scale=1.0
                )
                r_a = accpool.tile([128, N], f32, tag="r_a")
                rsc = accpool.tile([128, N], f32, tag="rsc")
                nc.vector.reciprocal_approx_accurate(out=r_a[:], in_=c1f[:], scratch=rsc[:])
                # x = (CE+CA) - CA*r_a
                x = accpool.tile([128, N], f32, tag="x")
                nc.scalar.activation(
                    out=x[:], in_=r_a[:], func=AFT.Copy, bias=CE + CA, scale=-CA
                )
                e1 = accpool.tile([128, N], f32, tag="e1")
                nc.scalar.activation(
                    out=e1[:], in_=ET_ps[jt][:], func=AFT.Copy, bias=1.0, scale=1.0
                )
                r_e = accpool.tile([128, N], f32, tag="r_e")
                rsc2 = accpool.tile([128, N], f32, tag="rsc2")
                nc.vector.reciprocal_approx_accurate(out=r_e[:], in_=e1[:], scratch=rsc2[:])
                cb = combpool.tile([128, N], f32, tag=f"comb{jt}", name=f"comb{jt}")
                nc.vector.scalar_tensor_tensor(
                    out=cb[:],
                    in0=r_e[:],
                    scalar=-CE,
                    in1=x[:],
                    op0=Alu.mult,
                    op1=Alu.add,
                )
                comb.append(cb)

            # min over the 4 chunks, then transpose + free-axis min over j
            nc.vector.tensor_tensor(comb[0][:], comb[0][:], comb[1][:], Alu.min)
            nc.vector.tensor_tensor(comb[2][:], comb[2][:], comb[3][:], Alu.min)
            nc.vector.tensor_tensor(comb[0][:], comb[0][:], comb[2][:], Alu.min)
            nov_p = []
            for h in range(2):
                tr = psB.tile([128, 128], f32, tag="tr", name=f"tr{h}")
                nc.tensor.transpose(tr[:], comb[0][:, h * 128 : (h + 1) * 128], eye_s[:])
                np_t = combpool.tile([128, 1], f32, tag=f"novp{h}", name=f"novp{h}")
                nc.vector.tensor_reduce(out=np_t[:], in_=tr[:], axis=X, op=Alu.min)
                nov_p.append(np_t)

            # ---- uni (gen x gen, my 32 j columns): [128i, 32j] layout -----
            uET_ps = [psC.tile([128, US], f32, tag="u", name=f"uET{k}") for k in range(2)]
            for j in range(US):
                ab = apool.tile([DE, N], bf16, tag="uab")
                nc.scalar.activation(
                    out=ab[:],
                    in_=Acs[:],
                    func=AFT.Abs,
                    bias=McsN[:, j : j + 1],
                    scale=1.0,
                )
                for ch in range(2):
                    nc.tensor.matmul(
                        uET_ps[ch][:, j : j + 1],
                        ab[:, ch * 128 : (ch + 1) * 128],
                        ones_e[:],
                        start=True,
                        stop=True,
                    )
            uni_p = []
            for ch in range(2):
                uacc = cheb_tree(
                    pmpool,
                    US,
                    lambda d: mrep[:, d * US : (d + 1) * US],
                    lambda d, ch=ch: ga_s[ch][:, d : d + 1],
                    f"upm{ch}",
                )
                uc1 = wpool.tile([128, US], f32, tag=f"uc1{ch}", name=f"uc1{ch}")
                nc.vector.tensor_scalar(
                    out=uc1[:], in0=uacc[:], scalar1=1.0, scalar2=None, op0=Alu.add
                )
                ur_a = wpool.tile([128, US], f32, tag=f"ura{ch}", name=f"ura{ch}")
                ursc = wpool.tile([128, US], f32, tag=f"ursc{ch}", name=f"ursc{ch}")
                nc.vector.reciprocal_approx_accurate(out=ur_a[:], in_=uc1[:], scratch=ursc[:])
                ux = wpool.tile([128, US], f32, tag=f"ux{ch}", name=f"ux{ch}")
                nc.vector.tensor_scalar(
                    out=ux[:],
                    in0=ur_a[:],
                    scalar1=-CA,
                    scalar2=CE + CA,
                    op0=Alu.mult,
                    op1=Alu.add,
                )
                ue1 = wpool.tile([128, US], f32, tag=f"ue1{ch}", name=f"ue1{ch}")
                nc.scalar.activation(
                    out=ue1[:], in_=uET_ps[ch][:], func=AFT.Copy, bias=1.0, scale=1.0
                )
                ur_e = wpool.tile([128, US], f32, tag=f"ure{ch}", name=f"ure{ch}")
                ursc2 = wpool.tile([128, US], f32, tag=f"ursc2{ch}", name=f"ursc2{ch}")
                nc.vector.reciprocal_approx_accurate(out=ur_e[:], in_=ue1[:], scratch=ursc2[:])
                ucomb = wpool.tile([128, US], f32, tag=f"ucomb{ch}", name=f"ucomb{ch}")
                nc.vector.scalar_tensor_tensor(
                    out=ucomb[:],
                    in0=ur_e[:],
                    scalar=-CE,
                    in1=ux[:],
                    op0=Alu.mult,
                    op1=Alu.add,
                )
                up = wpool.tile([128, 1], f32, tag=f"up{ch}", name=f"up{ch}")
                nc.vector.tensor_reduce(out=up[:], in_=ucomb[:], axis=X, op=Alu.add)
                uni_p.append(up)

            # ---------------- allgather partials ---------------------------
            cin = dpool.tile([1, 4 * 128], f32)
            nc.sync.dma_start(cin[:, 0:128], nov_p[0][:])
            nc.sync.dma_start(cin[:, 128:256], nov_p[1][:])
            nc.sync.dma_start(cin[:, 256:384], uni_p[0][:])
            nc.sync.dma_start(cin[:, 384:512], uni_p[1][:])
            cout = dpool.tile([NC, 4 * 128], f32)
            nc.gpsimd.collective_compute(
                "AllGather",
                Alu.bypass,
                replica_groups=[list(range(NC))],
                ins=[cin.opt()],
                outs=[cout.opt()],
            )
            nov_cat = wpool.tile([1, NC * N], f32)
            nc.sync.dma_start(nov_cat[:], cout[:, 0:N])
            uni_cat = wpool.tile([1, NC * N], f32)
            nc.sync.dma_start(uni_cat[:], cout[:, N : 2 * N])
            # reduce over the 8 core-chunks: view [1, (c n)] as [1, n, c]
            nov_r = wpool.tile([1, N], f32)
            nc.vector.tensor_reduce(
                out=nov_r[:],
                in_=nov_cat.rearrange("p (c n) -> p n c", c=NC),
                axis=X,
                op=Alu.min,
            )
            uni_r = wpool.tile([1, N], f32)
            nc.vector.tensor_reduce(
                out=uni_r[:],
                in_=uni_cat.rearrange("p (c n) -> p n c", c=NC),
                axis=X,
                op=Alu.add,
            )

            # ---------------- final ----------------------------------------
            stab = wpool.tile([1, N], f32)
            nc.vector.tensor_scalar(
                out=stab[:],
                in0=eah_s[:],
                scalar1=-1.0 / EAH_SCALE,
                scalar2=1.0,
                op0=Alu.mult,
                op1=Alu.add,
            )
            nc.vector.tensor_scalar(
                out=stab[:],
                in0=stab[:],
                scalar1=0.0,
                scalar2=1.0,
                op0=Alu.max,
                op1=Alu.min,
            )
            s1 = wpool.tile([1, N], f32)
            nc.vector.tensor_tensor(s1[:], stab[:], nov_r[:], Alu.mult)
            s2 = wpool.tile([1, N], f32)
            nc.vector.tensor_scalar(
                out=s2[:],
                in0=uni_r[:],
                scalar1=1.0 / (N - 1),
                scalar2=None,
                op0=Alu.mult,
            )
            sc = wpool.tile([1, N], f32)
            nc.vector.tensor_tensor(sc[:], s1[:], s2[:], Alu.mult)
            nc.sync.dma_start(out[:], sc[:])
            nc.sync.dma_start(dbg_nov[:], nov_r[:])
            nc.sync.dma_start(dbg_uni[:], uni_r[:])
            nc.sync.dma_start(dbg_stab[:], stab[:])

    nc.compile()
    return nc


def _get_nc():
    if "nc" not in _CACHE:
        _CACHE["nc"] = _build()
    return _CACHE["nc"]


def _in_maps(inputs):
    ge = np.ascontiguousarray(inputs["gen_embs_elmd"], dtype=np.float32)
    ga = np.ascontiguousarray(inputs["gen_embs_amd"], dtype=np.float32)
    re = np.ascontiguousarray(inputs["ref_embs_elmd"], dtype=np.float32)
    ra = np.ascontiguousarray(inputs["ref_embs_amd"], dtype=np.float32)
    eah = np.ascontiguousarray(inputs["e_above_hull"], dtype=np.float32).reshape(1, N)
    tri = np.triu(np.ones((DE, DE), dtype=np.float32))
    eye = np.eye(128, dtype=np.float32)
    maps = []
    for c in range(NC):
        maps.append(
            {
                "ge": ge,
                "ga": ga,
                "re": np.ascontiguousarray(re[c * MS : (c + 1) * MS]),
                "ra": np.ascontiguousarray(ra[c * MS : (c + 1) * MS]),
                "mge": np.ascontiguousarray(ge[c * US : (c + 1) * US]),
                "mga": np.ascontiguousarray(ga[c * US : (c + 1) * US]),
                "eah": eah,
                "tri": tri,
                "eye": eye,
            }
        )
    return maps


def kernel(**inputs) -> np.ndarray:
    from concourse.bass_utils import run_bass_kernel_spmd

    nc = _get_nc()
    res = run_bass_kernel_spmd(nc, _in_maps(inputs), list(range(NC)))
    return np.asarray(res.results[0]["scores"], dtype=np.float32).reshape(N)


# revision 22
# speedup vs baseline: 1.1155x; 1.0038x over previous
"""CSUN reward kernel (retrieval_knn) on 8 Trainium2 NeuronCores.

scores[i] = stability(eah[i]) * uni[i] * nov[i]
  d_elmd(i,j) = sum_d |cumsum(a)_id - cumsum(b)_jd|   (L1 on cumsum transform)
  d_amd(i,j)  = max_d |a_id - b_jd|                   (Chebyshev)
  combine(e,c) = Ce*e/(1+e) + Ca*c/(1+c) = (Ce+Ca) - Ce/(1+e) - Ca/(1+c)
  uni[i] = sum_j combine(gen,gen) / (N-1)   nov[i] = min_j combine(gen,ref)

Sharding: ref set M=4096 split 512/core (nov), gen axis split 32/core (uni).
Per core, everything lands in [128 j-partitions, i-free] layout:
  - ELMD: ScalarE activation Abs(Bcs - Acs_i) per i -> [103d, j] abs tile ->
    PE matmul (abs tile as weights, ones as rhs) -> column i of psum [128j, i].
  - AMD: custom DVE op, running Chebyshev max over d on [128j, i-free]:
    acc = max(acc, |arep_d - ra_d|) in one instruction per d.
Combine + min-tree over chunks, PE transpose, free-axis min -> nov partials.
AllGather + strided-reduce combines partials across cores.
"""

import sys

if "/opt/trn_rl_repo" not in sys.path:
    sys.path.insert(0, "/opt/trn_rl_repo")

import numpy as np

N = 256          # gen structures
M = 4096         # ref structures
DE = 103         # elmd dim
DA = 100         # amd dim
NC = 8           # cores
MS = M // NC     # 512 ref per core
US = N // NC     # 32 gen per core (uni shard)
NJT = MS // 128  # 4 ref j-tiles per core

COEF_ELMD = float.fromhex("0x1.8d7d565a99f87p-1")
COEF_AMD = float.fromhex("0x1.ca0aa695981e5p-3")
EAH_SCALE = 0.4289

_CACHE = {}


def _register_dve_op(name, spec, subdim=False, perf_en=None):
    from concourse import dve_ops
    from concourse.dve_spec import lower, _has_src1
    from concourse.dve_uop import DveOpSpec

    for o in dve_ops.OPS:
        if o.name == name:
            return o
    row = max(dve_ops._SUB_OPCODE_FOR_NAME.values()) + 1
    assert row < 0x20
    dve_ops._SUB_OPCODE_FOR_NAME[name] = row
    shas = {}
    for ver in ("v3", "v4"):
        tmp = DveOpSpec(
            name=name, opcode=row, uops=lower(spec, ver=ver), rd1_en=_has_src1(spec)
        )
        shas[ver] = tmp.sha(ver)
    op = dve_ops.DveOp(name, spec, subdim=subdim, uops_sha=shas, perf_en=perf_en or {})
    dve_ops.OPS.append(op)
    dve_ops.CUSTOM_DVE_SPECS[name] = spec
    return op


def _get_ops():
    from concourse.dve_spec import Spec, Src0, Src1, C0, C1, maxx

    absdiff = _register_dve_op(
        "ABSDIFF_ANT",
        Spec(
            body=maxx(Src0 - C0, C0 - Src0),
            reference=lambda in0, in1, s0, s1, imm2: np.maximum(
                in0.astype(np.float32) - s0, s0 - in0.astype(np.float32)
            ).astype(np.float32),
        ),
    )
    pairmax = _register_dve_op(
        "PAIRMAX_ANT",
        Spec(
            body=maxx(maxx(Src0 - C0, C0 - Src0), maxx(Src1 - C1, C1 - Src1)),
            reference=lambda in0, in1, s0, s1, imm2: np.maximum(
                np.maximum(
                    in0.astype(np.float32) - s0, s0 - in0.astype(np.float32)
                ),
                np.maximum(
                    in1.astype(np.float32) - s1, s1 - in1.astype(np.float32)
                ),
            ).astype(np.float32),
        ),
    )
    chebacc = _register_dve_op(
        "CHEBACC_ANT",
        Spec(
            body=maxx(maxx(Src0 - C0, C0 - Src0), Src1),
            reference=lambda in0, in1, s0, s1, imm2: np.maximum(
                np.maximum(in0.astype(np.float32) - s0, s0 - in0.astype(np.float32)),
                in1.astype(np.float32),
            ).astype(np.float32),
        ),
    )
    return absdiff, chebacc, pairmax


def _build():
    from concourse import bacc, tile, mybir

    f32 = mybir.dt.float32
    bf16 = mybir.dt.float16  # fp16: same DVE perf modes, 8x finer ulp here
    Alu = mybir.AluOpType
    AFT = mybir.ActivationFunctionType
    X = mybir.AxisListType.X

    ABSDIFF, CHEBACC, PAIRMAX = _get_ops()

    nc = bacc.Bacc("TRN2", target_bir_lowering=False, debug=False, num_devices=NC)

    ge = nc.dram_tensor("ge", [N, DE], f32, kind="ExternalInput").ap()
    ga = nc.dram_tensor("ga", [N, DA], f32, kind="ExternalInput").ap()
    re = nc.dram_tensor("re", [MS, DE], f32, kind="ExternalInput").ap()
    ra = nc.dram_tensor("ra", [MS, DA], f32, kind="ExternalInput").ap()
    mge = nc.dram_tensor("mge", [US, DE], f32, kind="ExternalInput").ap()
    mga = nc.dram_tensor("mga", [US, DA], f32, kind="ExternalInput").ap()
    eah = nc.dram_tensor("eah", [1, N], f32, kind="ExternalInput").ap()
    tri = nc.dram_tensor("tri", [DE, DE], f32, kind="ExternalInput").ap()
    eye = nc.dram_tensor("eye", [128, 128], f32, kind="ExternalInput").ap()
    out = nc.dram_tensor("scores", [1, N], f32, kind="ExternalOutput").ap()
    dbg_nov = nc.dram_tensor("dbg_nov", [1, N], f32, kind="ExternalOutput").ap()
    dbg_uni = nc.dram_tensor("dbg_uni", [1, N], f32, kind="ExternalOutput").ap()
    dbg_stab = nc.dram_tensor("dbg_stab", [1, N], f32, kind="ExternalOutput").ap()

    CE, CA = COEF_ELMD, COEF_AMD

    with tile.TileContext(nc) as tc:
        with (
            tc.tile_pool(name="const", bufs=1) as cpool,
            tc.tile_pool(name="work", bufs=1) as wpool,
            tc.tile_pool(name="abs", bufs=4) as apool,
            tc.tile_pool(name="acc", bufs=2) as accpool,
            tc.tile_pool(name="pm", bufs=8) as pmpool,
            tc.tile_pool(name="comb", bufs=1) as combpool,
            tc.tile_pool(name="psA", bufs=4, space="PSUM") as psA,
            tc.tile_pool(name="psB", bufs=2, space="PSUM") as psB,
            tc.tile_pool(name="psC", bufs=2, space="PSUM") as psC,
            tc.tile_pool(name="dram", bufs=1, space="DRAM") as dpool,
        ):
            # ---------------- prep ----------------
            tri_s = cpool.tile([DE, DE], f32)
            nc.sync.dma_start(tri_s[:], tri[:])
            eye_s = cpool.tile([128, 128], f32)
            nc.sync.dma_start(eye_s[:], eye[:])

            # per-partition scalar sources (also reused as transpose sources)
            ra_f = []
            for jt in range(NJT):
                t_f = cpool.tile([128, DA], f32, tag=f"raf{jt}", name=f"raf{jt}")
                nc.sync.dma_start(t_f[:], ra[jt * 128 : (jt + 1) * 128, :])
                ra_f.append(t_f)
            ga_s = []
            for ch in range(2):
                t_f = cpool.tile([128, DA], f32, tag=f"gas{ch}", name=f"gas{ch}")
                nc.sync.dma_start(t_f[:], ga[ch * 128 : (ch + 1) * 128, :])
                ga_s.append(t_f)

            # raw row-major loads used only as PE-transpose sources
            ge_r = []
            for ch in range(2):
                t = wpool.tile([128, DE], f32, tag=f"ger{ch}", name=f"ger{ch}")
                nc.sync.dma_start(t[:], ge[ch * 128 : (ch + 1) * 128, :])
                ge_r.append(t)
            re_r = []
            for jt in range(NJT):
                t = wpool.tile([128, DE], f32, tag=f"rer{jt}", name=f"rer{jt}")
                nc.sync.dma_start(t[:], re[jt * 128 : (jt + 1) * 128, :])
                re_r.append(t)
            mge_r = wpool.tile([US, DE], f32)
            nc.sync.dma_start(mge_r[:], mge[:])
            mga_r = wpool.tile([US, DA], f32)
            nc.sync.dma_start(mga_r[:], mga[:])

            # transpose via PE (x.T @ I), evacuate PSUM on ScalarE
            geT = wpool.tile([DE, N], f32)
            reT = wpool.tile([DE, MS], f32)
            gaT = wpool.tile([DA, N], f32)
            mgeT = wpool.tile([DE, US], f32)
            mgaT = wpool.tile([DA, US], f32)
            prep_tr = [
                (ge_r[0], geT[:, 0:128]),
                (ge_r[1], geT[:, 128:256]),
                (re_r[0], reT[:, 0:128]),
                (re_r[1], reT[:, 128:256]),
                (re_r[2], reT[:, 256:384]),
                (re_r[3], reT[:, 384:512]),
                (ga_s[0], gaT[:, 0:128]),
                (ga_s[1], gaT[:, 128:256]),
            ]
            for k, (src, dst) in enumerate(prep_tr):
                dcols = src.shape[1]
                tr = psB.tile([dcols, 128], f32, tag="tr", name=f"ptr{k}")
                nc.tensor.transpose(tr[:], src[:], eye_s[:])
                nc.scalar.copy(out=dst, in_=tr[:])
            tr_m = psB.tile([DE, US], f32, tag="tr", name="ptr_mge")
            nc.tensor.transpose(tr_m[:], mge_r[:], eye_s[0:US, 0:US])
            nc.scalar.copy(out=mgeT[:], in_=tr_m[:])
            tr_g = psB.tile([DA, US], f32, tag="tr", name="ptr_mga")
            nc.tensor.transpose(tr_g[:], mga_r[:], eye_s[0:US, 0:US])
            nc.scalar.copy(out=mgaT[:], in_=tr_g[:])

            # cumsum along d via upper-tri matmul: cs^T[d,i] = sum_{e<=d} x^T[e,i]
            csA_ps = psA.tile([DE, N], f32, tag="big")
            nc.tensor.matmul(csA_ps[:], tri_s[:], geT[:], start=True, stop=True)
            csB_ps = psA.tile([DE, MS], f32, tag="big")
            nc.tensor.matmul(csB_ps[:], tri_s[:], reT[:], start=True, stop=True)
            csM_ps = psA.tile([DE, US], f32, tag="big")
            nc.tensor.matmul(csM_ps[:], tri_s[:], mgeT[:], start=True, stop=True)

            Acs = cpool.tile([DE, N], bf16)
            nc.vector.tensor_copy(Acs[:], csA_ps[:])
            AcsN = cpool.tile([DE, N], bf16)  # negated fp16 cumsum, ACT bias
            nc.vector.tensor_scalar(
                out=AcsN[:], in0=Acs[:], scalar1=-1.0, scalar2=None, op0=Alu.mult
            )
            Bcs = cpool.tile([DE, MS], bf16)
            nc.vector.tensor_copy(Bcs[:], csB_ps[:])
            # round through bf16 so |Acs_i - Mcs_j| is exactly 0 on the uni
            # diagonal (saturating e/(1+e) otherwise turns rounding noise
            # into a large spurious diagonal term)
            McsH = cpool.tile([DE, US], bf16)
            nc.vector.tensor_copy(McsH[:], csM_ps[:])
            McsN = cpool.tile([DE, US], bf16)
            nc.vector.tensor_scalar(
                out=McsN[:], in0=McsH[:], scalar1=-1.0, scalar2=None, op0=Alu.mult
            )

            # replicated gen_amd^T, d-major: arep[p, d*N+i] = ga[i, d]
            gaT_bf = wpool.tile([DA, N], bf16)
            nc.vector.tensor_copy(gaT_bf[:], gaT[:])
            flat_d = dpool.tile([1, DA * N], bf16)
            nc.sync.dma_start(flat_d[:], gaT_bf[:])
            arep = cpool.tile([128, DA * N], bf16)
            DCH = 20  # d's per chunk: replicate via 0-stride DMA reads
            for c0 in range(0, DA, DCH):
                nc.sync.dma_start(
                    arep[:, c0 * N : (c0 + DCH) * N],
                    flat_d[:, c0 * N : (c0 + DCH) * N].partition_broadcast(128),
                )

            # replicated my-gen amd^T for uni: mrep[p, d*US+j] = mga[j, d]
            mgaT_bf = wpool.tile([DA, US], bf16)
            nc.vector.tensor_copy(mgaT_bf[:], mgaT[:])
            mflat_d = dpool.tile([1, DA * US], bf16)
            nc.sync.dma_start(mflat_d[:], mgaT_bf[:])
            mrep = cpool.tile([128, DA * US], bf16)
            nc.sync.dma_start(mrep[:], mflat_d.partition_broadcast(128))

            eah_s = cpool.tile([1, N], f32)
            nc.sync.dma_start(eah_s[:], eah[:])

            ones_e = cpool.tile([DE, 1], bf16)
            nc.vector.memset(ones_e[:], 1.0)

            def cheb_tree(pool, width, in_of, s_of, tag):
                # pairwise |a-b| max over d via PAIRMAX + streaming binary
                # merge tree of fp16 tensor_tensor(max); max ~log2 live tiles
                stack = []  # (level, tile)
                for k in range(DA // 2):
                    t = pool.tile([128, width], bf16, tag=tag, name=f"{tag}_{k}")
                    nc.vector._custom_dve(
                        PAIRMAX,
                        out=t[:],
                        in0=in_of(2 * k),
                        in1=in_of(2 * k + 1),
                        s0=s_of(2 * k),
                        s1=s_of(2 * k + 1),
                    )
                    lvl = 0
                    while stack and stack[-1][0] == lvl:
                        _, prev = stack.pop()
                        nc.vector.tensor_tensor(prev[:], prev[:], t[:], Alu.max)
                        t = prev
                        lvl += 1
                    stack.append((lvl, t))
                while len(stack) > 1:
                    _, a = stack.pop()
                    _, b = stack.pop()
                    nc.vector.tensor_tensor(b[:], b[:], a[:], Alu.max)
                    stack.append((99, b))
                return stack[0][1]

            uET_ps = [psC.tile([128, US], f32, tag="u", name=f"uET{k}") for k in range(2)]
            for j in range(US):
                ab = apool.tile([DE, N], bf16, tag="uab")
                nc.scalar.activation(
                    out=ab[:],
                    in_=Acs[:],
                    func=AFT.Abs,
                    bias=McsN[:, j : j + 1],
                    scale=1.0,
                )
                for ch in range(2):
                    nc.tensor.matmul(
                        uET_ps[ch][:, j : j + 1],
                        ab[:, ch * 128 : (ch + 1) * 128],
                        ones_e[:],
                        start=True,
                        stop=True,
                    )
            uni_p = []
            for ch in range(2):
                uacc = cheb_tree(
                    pmpool,
                    US,
                    lambda d: mrep[:, d * US : (d + 1) * US],
                    lambda d, ch=ch: ga_s[ch][:, d : d + 1],
                    f"upm{ch}",
                )
                uc1 = wpool.tile([128, US], f32, tag=f"uc1{ch}", name=f"uc1{ch}")
                nc.vector.tensor_scalar(
                    out=uc1[:], in0=uacc[:], scalar1=1.0, scalar2=None, op0=Alu.add
                )
                ur_a = wpool.tile([128, US], f32, tag=f"ura{ch}", name=f"ura{ch}")
                ursc = wpool.tile([128, US], f32, tag=f"ursc{ch}", name=f"ursc{ch}")
                nc.vector.reciprocal_approx_accurate(out=ur_a[:], in_=uc1[:], scratch=ursc[:])
                ux = wpool.tile([128, US], f32, tag=f"ux{ch}", name=f"ux{ch}")
                nc.vector.tensor_scalar(
                    out=ux[:],
                    in0=ur_a[:],
                    scalar1=-CA,
                    scalar2=CE + CA,
                    op0=Alu.mult,
                    op1=Alu.add,
                )
                ue1 = wpool.tile([128, US], f32, tag=f"ue1{ch}", name=f"ue1{ch}")
                nc.scalar.activation(
                    out=ue1[:], in_=uET_ps[ch][:], func=AFT.Copy, bias=1.0, scale=1.0
                )
                ur_e = wpool.tile([128, US], f32, tag=f"ure{ch}", name=f"ure{ch}")
                ursc2 = wpool.tile([128, US], f32, tag=f"ursc2{ch}", name=f"ursc2{ch}")
                nc.vector.reciprocal_approx_accurate(out=ur_e[:], in_=ue1[:], scratch=ursc2[:])
                ucomb = wpool.tile([128, US], f32, tag=f"ucomb{ch}", name=f"ucomb{ch}")
                nc.vector.scalar_tensor_tensor(
                    out=ucomb[:],
                    in0=ur_e[:],
                    scalar=-CE,
                    in1=ux[:],
                    op0=Alu.mult,
                    op1=Alu.add,
                )
                up = wpool.tile([128, 1], f32, tag=f"up{ch}", name=f"up{ch}")
                nc.vector.tensor_reduce(out=up[:], in_=ucomb[:], axis=X, op=Alu.add)
                uni_p.append(up)

            cin_u = dpool.tile([1, N], f32)
            nc.sync.dma_start(cin_u[:, 0:128], uni_p[0][:])
            nc.sync.dma_start(cin_u[:, 128:256], uni_p[1][:])
            cout_u = dpool.tile([NC, N], f32)
            nc.gpsimd.collective_compute(
                "AllGather",
                Alu.bypass,
                replica_groups=[list(range(NC))],
                ins=[cin_u.opt()],
                outs=[cout_u.opt()],
            )
            uni_cat = wpool.tile([1, NC * N], f32)
            nc.sync.dma_start(uni_cat[:], cout_u[:])
            uni_r = wpool.tile([1, N], f32)
            nc.vector.tensor_reduce(
                out=uni_r[:],
                in_=uni_cat.rearrange("p (c n) -> p n c", c=NC),
                axis=X,
                op=Alu.add,
            )

            # ---- ELMD nov via min identity: |a-b| = a+b-2*min(a,b) ------
            # ET[jt][j, i] = sum_d min(A_id, B_jd); SAB[jt][j, i] = 1+SA_i+SB_j
            # row/col sums SA [1,N], SB [1,MS] via PE ones-matmuls
            ET_ps = [
                psA.tile([128, N], f32, tag="big", name=f"ET{k}") for k in range(NJT)
            ]
            for i in range(N):
                ab = apool.tile([DE, MS], bf16, tag="ab")
                nc.scalar.activation(
                    out=ab[:],
                    in_=Bcs[:],
                    func=AFT.Abs,
                    bias=AcsN[:, i : i + 1],
                    scale=1.0,
                )
                for jt in range(NJT):
                    nc.tensor.matmul(
                        ET_ps[jt][:, i : i + 1],
                        ab[:, jt * 128 : (jt + 1) * 128],
                        ones_e[:],
                        start=True,
                        stop=True,
                    )

            # ---- AMD nov + combine per j-tile -> comb[jt] [128j, 256i] ----
            accs = []
            for jt in range(NJT):
                accs.append(
                    cheb_tree(
                        pmpool,
                        N,
                        lambda d: arep[:, d * N : (d + 1) * N],
                        lambda d, jt=jt: ra_f[jt][:, d : d + 1],
                        f"pm{jt}",
                    )
                )
            comb = []
            for jt in range(NJT):
                # affine stages on ScalarE (ACT), reciprocals on DVE
                c1f = accpool.tile([128, N], f32, tag="c1f")
                nc.scalar.activation(
                    out=c1f[:], in_=accs[jt][:], func=AFT.Copy, bias=1.0, scale=1.0
                )
                r_a = accpool.tile([128, N], f32, tag="r_a")
                rsc = accpool.tile([128, N], f32, tag="rsc")
                nc.vector.reciprocal_approx_accurate(out=r_a[:], in_=c1f[:], scratch=rsc[:])
                # x = (CE+CA) - CA*r_a
                x = accpool.tile([128, N], f32, tag="x")
                nc.scalar.activation(
                    out=x[:], in_=r_a[:], func=AFT.Copy, bias=CE + CA, scale=-CA
                )
                e1 = accpool.tile([128, N], f32, tag="e1")
                nc.scalar.activation(
                    out=e1[:], in_=ET_ps[jt][:], func=AFT.Copy, bias=1.0, scale=1.0
                )
                r_e = accpool.tile([128, N], f32, tag="r_e")
                rsc2 = accpool.tile([128, N], f32, tag="rsc2")
                nc.vector.reciprocal_approx_accurate(out=r_e[:], in_=e1[:], scratch=rsc2[:])
                cb = combpool.tile([128, N], f32, tag=f"comb{jt}", name=f"comb{jt}")
                nc.vector.scalar_tensor_tensor(
                    out=cb[:],
                    in0=r_e[:],
                    scalar=-CE,
                    in1=x[:],
                    op0=Alu.mult,
                    op1=Alu.add,
                )
                comb.append(cb)

            # min over the 4 chunks, then transpose + free-axis min over j
            nc.vector.tensor_tensor(comb[0][:], comb[0][:], comb[1][:], Alu.min)
            nc.vector.tensor_tensor(comb[2][:], comb[2][:], comb[3][:], Alu.min)
            nc.vector.tensor_tensor(comb[0][:], comb[0][:], comb[2][:], Alu.min)
            nov_p = []
            for h in range(2):
                tr = psB.tile([128, 128], f32, tag="tr", name=f"tr{h}")
                nc.tensor.transpose(tr[:], comb[0][:, h * 128 : (h + 1) * 128], eye_s[:])
                np_t = combpool.tile([128, 1], f32, tag=f"novp{h}", name=f"novp{h}")
                nc.vector.tensor_reduce(out=np_t[:], in_=tr[:], axis=X, op=Alu.min)
                nov_p.append(np_t)

            # ---- uni (gen x gen, my 32 j columns): [128i, 32j] layout -----
            # ---------------- nov allgather (tail) --------------------------
            cin = dpool.tile([1, N], f32)
            nc.sync.dma_start(cin[:, 0:128], nov_p[0][:])
            nc.sync.dma_start(cin[:, 128:256], nov_p[1][:])
            cout = dpool.tile([NC, N], f32)
            nc.gpsimd.collective_compute(
                "AllGather",
                Alu.bypass,
                replica_groups=[list(range(NC))],
                ins=[cin.opt()],
                outs=[cout.opt()],
            )
            nov_cat = wpool.tile([1, NC * N], f32)
            nc.sync.dma_start(nov_cat[:], cout[:])
            nov_r = wpool.tile([1, N], f32)
            nc.vector.tensor_reduce(
                out=nov_r[:],
                in_=nov_cat.rearrange("p (c n) -> p n c", c=NC),
                axis=X,
                op=Alu.min,
            )

            # ---------------- final ----------------------------------------
            stab = wpool.tile([1, N], f32)
            nc.vector.tensor_scalar(
                out=stab[:],
                in0=eah_s[:],
                scalar1=-1.0 / EAH_SCALE,
                scalar2=1.0,
                op0=Alu.mult,
                op1=Alu.add,
            )
            nc.vector.tensor_scalar(
                out=stab[:],
                in0=stab[:],
                scalar1=0.0,
                scalar2=1.0,
                op0=Alu.max,
                op1=Alu.min,
            )
            s1 = wpool.tile([1, N], f32)
            nc.vector.tensor_tensor(s1[:], stab[:], nov_r[:], Alu.mult)
            s2 = wpool.tile([1, N], f32)
            nc.vector.tensor_scalar(
                out=s2[:],
                in0=uni_r[:],
                scalar1=1.0 / (N - 1),
                scalar2=None,
                op0=Alu.mult,
            )
            sc = wpool.tile([1, N], f32)
            nc.vector.tensor_tensor(sc[:], s1[:], s2[:], Alu.mult)
            nc.sync.dma_start(out[:], sc[:])
            nc.sync.dma_start(dbg_nov[:], nov_r[:])
            nc.sync.dma_start(dbg_uni[:], uni_r[:])
            nc.sync.dma_start(dbg_stab[:], stab[:])

    nc.compile()
    return nc


def _get_nc():
    if "nc" not in _CACHE:
        _CACHE["nc"] = _build()
    return _CACHE["nc"]


def _in_maps(inputs):
    ge = np.ascontiguousarray(inputs["gen_embs_elmd"], dtype=np.float32)
    ga = np.ascontiguousarray(inputs["gen_embs_amd"], dtype=np.float32)
    re = np.ascontiguousarray(inputs["ref_embs_elmd"], dtype=np.float32)
    ra = np.ascontiguousarray(inputs["ref_embs_amd"], dtype=np.float32)
    eah = np.ascontiguousarray(inputs["e_above_hull"], dtype=np.float32).reshape(1, N)
    tri = np.triu(np.ones((DE, DE), dtype=np.float32))
    eye = np.eye(128, dtype=np.float32)
    maps = []
    for c in range(NC):
        maps.append(
            {
                "ge": ge,
                "ga": ga,
                "re": np.ascontiguousarray(re[c * MS : (c + 1) * MS]),
                "ra": np.ascontiguousarray(ra[c * MS : (c + 1) * MS]),
                "mge": np.ascontiguousarray(ge[c * US : (c + 1) * US]),
                "mga": np.ascontiguousarray(ga[c * US : (c + 1) * US]),
                "eah": eah,
                "tri": tri,
                "eye": eye,
            }
        )
    return maps


def kernel(**inputs) -> np.ndarray:
    from concourse.bass_utils import run_bass_kernel_spmd

    nc = _get_nc()
    res = run_bass_kernel_spmd(nc, _in_maps(inputs), list(range(NC)))
    return np.asarray(res.results[0]["scores"], dtype=np.float32).reshape(N)


# revision 23
# speedup vs baseline: 1.2205x; 1.0941x over previous
"""CSUN reward kernel (retrieval_knn) on 8 Trainium2 NeuronCores.

scores[i] = stability(eah[i]) * uni[i] * nov[i]
  d_elmd(i,j) = sum_d |cumsum(a)_id - cumsum(b)_jd|   (L1 on cumsum transform)
  d_amd(i,j)  = max_d |a_id - b_jd|                   (Chebyshev)
  combine(e,c) = Ce*e/(1+e) + Ca*c/(1+c) = (Ce+Ca) - Ce/(1+e) - Ca/(1+c)
  uni[i] = sum_j combine(gen,gen) / (N-1)   nov[i] = min_j combine(gen,ref)

Sharding: ref set M=4096 split 512/core (nov), gen axis split 32/core (uni).
Per core, everything lands in [128 j-partitions, i-free] layout:
  - ELMD: ScalarE activation Abs(Bcs - Acs_i) per i -> [103d, j] abs tile ->
    PE matmul (abs tile as weights, ones as rhs) -> column i of psum [128j, i].
  - AMD: custom DVE op, running Chebyshev max over d on [128j, i-free]:
    acc = max(acc, |arep_d - ra_d|) in one instruction per d.
Combine + min-tree over chunks, PE transpose, free-axis min -> nov partials.
AllGather + strided-reduce combines partials across cores.
"""

import sys

if "/opt/trn_rl_repo" not in sys.path:
    sys.path.insert(0, "/opt/trn_rl_repo")

import numpy as np

N = 256          # gen structures
M = 4096         # ref structures
DE = 103         # elmd dim
DA = 100         # amd dim
NC = 8           # cores
MS = M // NC     # 512 ref per core
US = N // NC     # 32 gen per core (uni shard)
NJT = MS // 128  # 4 ref j-tiles per core

COEF_ELMD = float.fromhex("0x1.8d7d565a99f87p-1")
COEF_AMD = float.fromhex("0x1.ca0aa695981e5p-3")
EAH_SCALE = 0.4289

_CACHE = {}


def _register_dve_op(name, spec, subdim=False, perf_en=None):
    from concourse import dve_ops
    from concourse.dve_spec import lower, _has_src1
    from concourse.dve_uop import DveOpSpec

    for o in dve_ops.OPS:
        if o.name == name:
            return o
    row = max(dve_ops._SUB_OPCODE_FOR_NAME.values()) + 1
    assert row < 0x20
    dve_ops._SUB_OPCODE_FOR_NAME[name] = row
    shas = {}
    for ver in ("v3", "v4"):
        tmp = DveOpSpec(
            name=name, opcode=row, uops=lower(spec, ver=ver), rd1_en=_has_src1(spec)
        )
        shas[ver] = tmp.sha(ver)
    op = dve_ops.DveOp(name, spec, subdim=subdim, uops_sha=shas, perf_en=perf_en or {})
    dve_ops.OPS.append(op)
    dve_ops.CUSTOM_DVE_SPECS[name] = spec
    return op


def _get_ops():
    from concourse.dve_spec import Spec, Src0, Src1, C0, C1, maxx

    absdiff = _register_dve_op(
        "ABSDIFF_ANT",
        Spec(
            body=maxx(Src0 - C0, C0 - Src0),
            reference=lambda in0, in1, s0, s1, imm2: np.maximum(
                in0.astype(np.float32) - s0, s0 - in0.astype(np.float32)
            ).astype(np.float32),
        ),
    )
    pairmax = _register_dve_op(
        "PAIRMAX_ANT",
        Spec(
            body=maxx(maxx(Src0 - C0, C0 - Src0), maxx(Src1 - C1, C1 - Src1)),
            reference=lambda in0, in1, s0, s1, imm2: np.maximum(
                np.maximum(
                    in0.astype(np.float32) - s0, s0 - in0.astype(np.float32)
                ),
                np.maximum(
                    in1.astype(np.float32) - s1, s1 - in1.astype(np.float32)
                ),
            ).astype(np.float32),
        ),
    )
    chebacc = _register_dve_op(
        "CHEBACC_ANT",
        Spec(
            body=maxx(maxx(Src0 - C0, C0 - Src0), Src1),
            reference=lambda in0, in1, s0, s1, imm2: np.maximum(
                np.maximum(in0.astype(np.float32) - s0, s0 - in0.astype(np.float32)),
                in1.astype(np.float32),
            ).astype(np.float32),
        ),
    )
    return absdiff, chebacc, pairmax


def _build():
    from concourse import bacc, tile, mybir

    f32 = mybir.dt.float32
    bf16 = mybir.dt.float16  # fp16: same DVE perf modes, 8x finer ulp here
    Alu = mybir.AluOpType
    AFT = mybir.ActivationFunctionType
    X = mybir.AxisListType.X

    ABSDIFF, CHEBACC, PAIRMAX = _get_ops()

    nc = bacc.Bacc("TRN2", target_bir_lowering=False, debug=False, num_devices=NC)

    ge = nc.dram_tensor("ge", [N, DE], f32, kind="ExternalInput").ap()
    ga = nc.dram_tensor("ga", [N, DA], f32, kind="ExternalInput").ap()
    re = nc.dram_tensor("re", [MS, DE], f32, kind="ExternalInput").ap()
    ra = nc.dram_tensor("ra", [MS, DA], f32, kind="ExternalInput").ap()
    mge = nc.dram_tensor("mge", [US, DE], f32, kind="ExternalInput").ap()
    mga = nc.dram_tensor("mga", [US, DA], f32, kind="ExternalInput").ap()
    eah = nc.dram_tensor("eah", [1, N], f32, kind="ExternalInput").ap()
    tri = nc.dram_tensor("tri", [DE, DE], f32, kind="ExternalInput").ap()
    eye = nc.dram_tensor("eye", [128, 128], f32, kind="ExternalInput").ap()
    out = nc.dram_tensor("scores", [1, N], f32, kind="ExternalOutput").ap()
    dbg_nov = nc.dram_tensor("dbg_nov", [1, N], f32, kind="ExternalOutput").ap()
    dbg_uni = nc.dram_tensor("dbg_uni", [1, N], f32, kind="ExternalOutput").ap()
    dbg_stab = nc.dram_tensor("dbg_stab", [1, N], f32, kind="ExternalOutput").ap()

    CE, CA = COEF_ELMD, COEF_AMD

    with tile.TileContext(nc) as tc:
        with (
            tc.tile_pool(name="const", bufs=1) as cpool,
            tc.tile_pool(name="work", bufs=1) as wpool,
            tc.tile_pool(name="abs", bufs=4) as apool,
            tc.tile_pool(name="acc", bufs=2) as accpool,
            tc.tile_pool(name="pm", bufs=8) as pmpool,
            tc.tile_pool(name="comb", bufs=1) as combpool,
            tc.tile_pool(name="psA", bufs=4, space="PSUM") as psA,
            tc.tile_pool(name="psB", bufs=2, space="PSUM") as psB,
            tc.tile_pool(name="psC", bufs=2, space="PSUM") as psC,
            tc.tile_pool(name="dram", bufs=1, space="DRAM") as dpool,
        ):
            # ---------------- prep ----------------
            tri_s = cpool.tile([DE, DE], f32)
            nc.sync.dma_start(tri_s[:], tri[:])
            eye_s = cpool.tile([128, 128], f32)
            nc.sync.dma_start(eye_s[:], eye[:])

            # per-partition scalar sources (also reused as transpose sources)
            ra_f = []
            for jt in range(NJT):
                t_f = cpool.tile([128, DA], f32, tag=f"raf{jt}", name=f"raf{jt}")
                nc.sync.dma_start(t_f[:], ra[jt * 128 : (jt + 1) * 128, :])
                ra_f.append(t_f)
            ga_s = []
            for ch in range(2):
                t_f = cpool.tile([128, DA], f32, tag=f"gas{ch}", name=f"gas{ch}")
                nc.sync.dma_start(t_f[:], ga[ch * 128 : (ch + 1) * 128, :])
                ga_s.append(t_f)

            # raw row-major loads used only as PE-transpose sources
            ge_r = []
            for ch in range(2):
                t = wpool.tile([128, DE], f32, tag=f"ger{ch}", name=f"ger{ch}")
                nc.sync.dma_start(t[:], ge[ch * 128 : (ch + 1) * 128, :])
                ge_r.append(t)
            re_r = []
            for jt in range(NJT):
                t = wpool.tile([128, DE], f32, tag=f"rer{jt}", name=f"rer{jt}")
                nc.sync.dma_start(t[:], re[jt * 128 : (jt + 1) * 128, :])
                re_r.append(t)
            mge_r = wpool.tile([US, DE], f32)
            nc.sync.dma_start(mge_r[:], mge[:])
            mga_r = wpool.tile([US, DA], f32)
            nc.sync.dma_start(mga_r[:], mga[:])

            # transpose via PE (x.T @ I), evacuate PSUM on ScalarE
            geT = wpool.tile([DE, N], f32)
            reT = wpool.tile([DE, MS], f32)
            gaT = wpool.tile([DA, N], f32)
            mgeT = wpool.tile([DE, US], f32)
            mgaT = wpool.tile([DA, US], f32)
            prep_tr = [
                (ge_r[0], geT[:, 0:128]),
                (ge_r[1], geT[:, 128:256]),
                (re_r[0], reT[:, 0:128]),
                (re_r[1], reT[:, 128:256]),
                (re_r[2], reT[:, 256:384]),
                (re_r[3], reT[:, 384:512]),
                (ga_s[0], gaT[:, 0:128]),
                (ga_s[1], gaT[:, 128:256]),
            ]
            for k, (src, dst) in enumerate(prep_tr):
                dcols = src.shape[1]
                tr = psB.tile([dcols, 128], f32, tag="tr", name=f"ptr{k}")
                nc.tensor.transpose(tr[:], src[:], eye_s[:])
                nc.scalar.copy(out=dst, in_=tr[:])
            tr_m = psB.tile([DE, US], f32, tag="tr", name="ptr_mge")
            nc.tensor.transpose(tr_m[:], mge_r[:], eye_s[0:US, 0:US])
            nc.scalar.copy(out=mgeT[:], in_=tr_m[:])
            tr_g = psB.tile([DA, US], f32, tag="tr", name="ptr_mga")
            nc.tensor.transpose(tr_g[:], mga_r[:], eye_s[0:US, 0:US])
            nc.scalar.copy(out=mgaT[:], in_=tr_g[:])

            # cumsum along d via upper-tri matmul: cs^T[d,i] = sum_{e<=d} x^T[e,i]
            csA_ps = psA.tile([DE, N], f32, tag="big")
            nc.tensor.matmul(csA_ps[:], tri_s[:], geT[:], start=True, stop=True)
            csB_ps = psA.tile([DE, MS], f32, tag="big")
            nc.tensor.matmul(csB_ps[:], tri_s[:], reT[:], start=True, stop=True)
            csM_ps = psA.tile([DE, US], f32, tag="big")
            nc.tensor.matmul(csM_ps[:], tri_s[:], mgeT[:], start=True, stop=True)

            Acs = cpool.tile([DE, N], bf16)
            nc.vector.tensor_copy(Acs[:], csA_ps[:])
            AcsN = cpool.tile([DE, N], bf16)  # negated fp16 cumsum, ACT bias
            nc.vector.tensor_scalar(
                out=AcsN[:], in0=Acs[:], scalar1=-1.0, scalar2=None, op0=Alu.mult
            )
            Bcs = cpool.tile([DE, MS], bf16)
            nc.vector.tensor_copy(Bcs[:], csB_ps[:])
            # round through bf16 so |Acs_i - Mcs_j| is exactly 0 on the uni
            # diagonal (saturating e/(1+e) otherwise turns rounding noise
            # into a large spurious diagonal term)
            McsH = cpool.tile([DE, US], bf16)
            nc.vector.tensor_copy(McsH[:], csM_ps[:])
            McsN = cpool.tile([DE, US], bf16)
            nc.vector.tensor_scalar(
                out=McsN[:], in0=McsH[:], scalar1=-1.0, scalar2=None, op0=Alu.mult
            )

            # replicated gen_amd^T, d-major: arep[p, d*N+i] = ga[i, d]
            gaT_bf = wpool.tile([DA, N], bf16)
            nc.vector.tensor_copy(gaT_bf[:], gaT[:])
            flat_d = dpool.tile([1, DA * N], bf16)
            nc.sync.dma_start(flat_d[:], gaT_bf[:])
            arep = cpool.tile([128, DA * N], bf16)
            DCH = 20  # d's per chunk: replicate via 0-stride DMA reads
            for c0 in range(0, DA, DCH):
                nc.sync.dma_start(
                    arep[:, c0 * N : (c0 + DCH) * N],
                    flat_d[:, c0 * N : (c0 + DCH) * N].partition_broadcast(128),
                )

            # replicated my-gen amd^T for uni: mrep[p, d*US+j] = mga[j, d]
            mgaT_bf = wpool.tile([DA, US], bf16)
            nc.vector.tensor_copy(mgaT_bf[:], mgaT[:])
            mflat_d = dpool.tile([1, DA * US], bf16)
            nc.sync.dma_start(mflat_d[:], mgaT_bf[:])
            mrep = cpool.tile([128, DA * US], bf16)
            nc.sync.dma_start(mrep[:], mflat_d.partition_broadcast(128))

            eah_s = cpool.tile([1, N], f32)
            nc.sync.dma_start(eah_s[:], eah[:])

            ones_e = cpool.tile([DE, 1], bf16)
            nc.vector.memset(ones_e[:], 1.0)

            def cheb_tree(pool, width, in_of, s_of, tag):
                # pairwise |a-b| max over d via PAIRMAX + streaming binary
                # merge tree of fp16 tensor_tensor(max); max ~log2 live tiles
                stack = []  # (level, tile)
                for k in range(DA // 2):
                    t = pool.tile([128, width], bf16, tag=tag, name=f"{tag}_{k}")
                    nc.vector._custom_dve(
                        PAIRMAX,
                        out=t[:],
                        in0=in_of(2 * k),
                        in1=in_of(2 * k + 1),
                        s0=s_of(2 * k),
                        s1=s_of(2 * k + 1),
                    )
                    lvl = 0
                    while stack and stack[-1][0] == lvl:
                        _, prev = stack.pop()
                        nc.vector.tensor_tensor(prev[:], prev[:], t[:], Alu.max)
                        t = prev
                        lvl += 1
                    stack.append((lvl, t))
                while len(stack) > 1:
                    _, a = stack.pop()
                    _, b = stack.pop()
                    nc.vector.tensor_tensor(b[:], b[:], a[:], Alu.max)
                    stack.append((99, b))
                return stack[0][1]

            uET_ps = [psC.tile([128, US], f32, tag="u", name=f"uET{k}") for k in range(2)]
            for j in range(US):
                ab = apool.tile([DE, N], bf16, tag="uab")
                nc.scalar.activation(
                    out=ab[:],
                    in_=Acs[:],
                    func=AFT.Abs,
                    bias=McsN[:, j : j + 1],
                    scale=1.0,
                )
                for ch in range(2):
                    nc.tensor.matmul(
                        uET_ps[ch][:, j : j + 1],
                        ab[:, ch * 128 : (ch + 1) * 128],
                        ones_e[:],
                        start=True,
                        stop=True,
                    )
            uni_p = []
            for ch in range(2):
                uacc = cheb_tree(
                    pmpool,
                    US,
                    lambda d: mrep[:, d * US : (d + 1) * US],
                    lambda d, ch=ch: ga_s[ch][:, d : d + 1],
                    f"upm{ch}",
                )
                uc1 = wpool.tile([128, US], f32, tag=f"uc1{ch}", name=f"uc1{ch}")
                nc.vector.tensor_scalar(
                    out=uc1[:], in0=uacc[:], scalar1=1.0, scalar2=None, op0=Alu.add
                )
                ur_a = wpool.tile([128, US], f32, tag=f"ura{ch}", name=f"ura{ch}")
                ursc = wpool.tile([128, US], f32, tag=f"ursc{ch}", name=f"ursc{ch}")
                nc.vector.reciprocal_approx_accurate(out=ur_a[:], in_=uc1[:], scratch=ursc[:])
                ux = wpool.tile([128, US], f32, tag=f"ux{ch}", name=f"ux{ch}")
                nc.vector.tensor_scalar(
                    out=ux[:],
                    in0=ur_a[:],
                    scalar1=-CA,
                    scalar2=CE + CA,
                    op0=Alu.mult,
                    op1=Alu.add,
                )
                ue1 = wpool.tile([128, US], f32, tag=f"ue1{ch}", name=f"ue1{ch}")
                nc.scalar.activation(
                    out=ue1[:], in_=uET_ps[ch][:], func=AFT.Copy, bias=1.0, scale=1.0
                )
                ur_e = wpool.tile([128, US], f32, tag=f"ure{ch}", name=f"ure{ch}")
                ursc2 = wpool.tile([128, US], f32, tag=f"ursc2{ch}", name=f"ursc2{ch}")
                nc.vector.reciprocal_approx_accurate(out=ur_e[:], in_=ue1[:], scratch=ursc2[:])
                ucomb = wpool.tile([128, US], f32, tag=f"ucomb{ch}", name=f"ucomb{ch}")
                nc.vector.scalar_tensor_tensor(
                    out=ucomb[:],
                    in0=ur_e[:],
                    scalar=-CE,
                    in1=ux[:],
                    op0=Alu.mult,
                    op1=Alu.add,
                )
                up = wpool.tile([128, 1], f32, tag=f"up{ch}", name=f"up{ch}")
                nc.vector.tensor_reduce(out=up[:], in_=ucomb[:], axis=X, op=Alu.add)
                uni_p.append(up)

            cin_u = dpool.tile([1, N], f32)
            nc.sync.dma_start(cin_u[:, 0:128], uni_p[0][:])
            nc.sync.dma_start(cin_u[:, 128:256], uni_p[1][:])
            cout_u = dpool.tile([NC, N], f32)
            nc.gpsimd.collective_compute(
                "AllGather",
                Alu.bypass,
                replica_groups=[list(range(NC))],
                ins=[cin_u.opt()],
                outs=[cout_u.opt()],
            )
            uni_cat = wpool.tile([1, NC * N], f32)
            nc.sync.dma_start(uni_cat[:], cout_u[:])
            uni_r = wpool.tile([1, N], f32)
            nc.vector.tensor_reduce(
                out=uni_r[:],
                in_=uni_cat.rearrange("p (c n) -> p n c", c=NC),
                axis=X,
                op=Alu.add,
            )

            # ---- ELMD nov via min identity: |a-b| = a+b-2*min(a,b) ------
            # ET[jt][j, i] = sum_d min(A_id, B_jd); SAB[jt][j, i] = 1+SA_i+SB_j
            # row/col sums SA [1,N], SB [1,MS] via PE ones-matmuls
            ET_ps = [
                psA.tile([128, N], f32, tag="big", name=f"ET{k}") for k in range(NJT)
            ]
            for i in range(N):
                ab = apool.tile([DE, MS], bf16, tag="ab")
                nc.scalar.activation(
                    out=ab[:],
                    in_=Bcs[:],
                    func=AFT.Abs,
                    bias=AcsN[:, i : i + 1],
                    scale=1.0,
                )
                for jt in range(NJT):
                    nc.tensor.matmul(
                        ET_ps[jt][:, i : i + 1],
                        ab[:, jt * 128 : (jt + 1) * 128],
                        ones_e[:],
                        start=True,
                        stop=True,
                    )

            # ---- AMD nov + combine per j-tile -> comb[jt] [128j, 256i] ----
            # all 4 j-tiles share one wide tile per pair so the merge tree
            # runs 4x-wide ops (fewer dispatches)
            stack = []
            for k in range(DA // 2):
                t = pmpool.tile([128, NJT * N], bf16, tag="pmS", name=f"pmS_{k}")
                for jt in range(NJT):
                    nc.vector._custom_dve(
                        PAIRMAX,
                        out=t[:, jt * N : (jt + 1) * N],
                        in0=arep[:, 2 * k * N : (2 * k + 1) * N],
                        in1=arep[:, (2 * k + 1) * N : (2 * k + 2) * N],
                        s0=ra_f[jt][:, 2 * k : 2 * k + 1],
                        s1=ra_f[jt][:, 2 * k + 1 : 2 * k + 2],
                    )
                lvl = 0
                while stack and stack[-1][0] == lvl:
                    _, prev = stack.pop()
                    nc.vector.tensor_tensor(prev[:], prev[:], t[:], Alu.max)
                    t = prev
                    lvl += 1
                stack.append((lvl, t))
            while len(stack) > 1:
                _, a = stack.pop()
                _, b = stack.pop()
                nc.vector.tensor_tensor(b[:], b[:], a[:], Alu.max)
                stack.append((99, b))
            acc_all = stack[0][1]
            accs = [acc_all[:, jt * N : (jt + 1) * N] for jt in range(NJT)]
            comb = []
            for jt in range(NJT):
                # affine stages on ScalarE (ACT), reciprocals on DVE
                c1f = accpool.tile([128, N], f32, tag="c1f")
                nc.scalar.activation(
                    out=c1f[:], in_=accs[jt][:], func=AFT.Copy, bias=1.0, scale=1.0
                )
                r_a = accpool.tile([128, N], f32, tag="r_a")
                rsc = accpool.tile([128, N], f32, tag="rsc")
                nc.vector.reciprocal_approx_accurate(out=r_a[:], in_=c1f[:], scratch=rsc[:])
                # x = (CE+CA) - CA*r_a
                x = accpool.tile([128, N], f32, tag="x")
                nc.scalar.activation(
                    out=x[:], in_=r_a[:], func=AFT.Copy, bias=CE + CA, scale=-CA
                )
                e1 = accpool.tile([128, N], f32, tag="e1")
                nc.scalar.activation(
                    out=e1[:], in_=ET_ps[jt][:], func=AFT.Copy, bias=1.0, scale=1.0
                )
                r_e = accpool.tile([128, N], f32, tag="r_e")
                rsc2 = accpool.tile([128, N], f32, tag="rsc2")
                nc.vector.reciprocal_approx_accurate(out=r_e[:], in_=e1[:], scratch=rsc2[:])
                cb = combpool.tile([128, N], f32, tag=f"comb{jt}", name=f"comb{jt}")
                nc.vector.scalar_tensor_tensor(
                    out=cb[:],
                    in0=r_e[:],
                    scalar=-CE,
                    in1=x[:],
                    op0=Alu.mult,
                    op1=Alu.add,
                )
                comb.append(cb)

            # min over the 4 chunks, then transpose + free-axis min over j
            nc.vector.tensor_tensor(comb[0][:], comb[0][:], comb[1][:], Alu.min)
            nc.vector.tensor_tensor(comb[2][:], comb[2][:], comb[3][:], Alu.min)
            nc.vector.tensor_tensor(comb[0][:], comb[0][:], comb[2][:], Alu.min)
            nov_p = []
            for h in range(2):
                tr = psB.tile([128, 128], f32, tag="tr", name=f"tr{h}")
                nc.tensor.transpose(tr[:], comb[0][:, h * 128 : (h + 1) * 128], eye_s[:])
                np_t = combpool.tile([128, 1], f32, tag=f"novp{h}", name=f"novp{h}")
                nc.vector.tensor_reduce(out=np_t[:], in_=tr[:], axis=X, op=Alu.min)
                nov_p.append(np_t)

            # ---- uni (gen x gen, my 32 j columns): [128i, 32j] layout -----
            # ---------------- nov allgather (tail) --------------------------
            cin = dpool.tile([1, N], f32)
            nc.sync.dma_start(cin[:, 0:128], nov_p[0][:])
            nc.sync.dma_start(cin[:, 128:256], nov_p[1][:])
            cout = dpool.tile([NC, N], f32)
            nc.gpsimd.collective_compute(
                "AllGather",
                Alu.bypass,
                replica_groups=[list(range(NC))],
                ins=[cin.opt()],
                outs=[cout.opt()],
            )
            nov_cat = wpool.tile([1, NC * N], f32)
            nc.sync.dma_start(nov_cat[:], cout[:])
            nov_r = wpool.tile([1, N], f32)
            nc.vector.tensor_reduce(
                out=nov_r[:],
                in_=nov_cat.rearrange("p (c n) -> p n c", c=NC),
                axis=X,
                op=Alu.min,
            )

            # ---------------- final ----------------------------------------
            stab = wpool.tile([1, N], f32)
            nc.vector.tensor_scalar(
                out=stab[:],
                in0=eah_s[:],
                scalar1=-1.0 / EAH_SCALE,
                scalar2=1.0,
                op0=Alu.mult,
                op1=Alu.add,
            )
            nc.vector.tensor_scalar(
                out=stab[:],
                in0=stab[:],
                scalar1=0.0,
                scalar2=1.0,
                op0=Alu.max,
                op1=Alu.min,
            )
            s1 = wpool.tile([1, N], f32)
            nc.vector.tensor_tensor(s1[:], stab[:], nov_r[:], Alu.mult)
            s2 = wpool.tile([1, N], f32)
            nc.vector.tensor_scalar(
                out=s2[:],
                in0=uni_r[:],
                scalar1=1.0 / (N - 1),
                scalar2=None,
                op0=Alu.mult,
            )
            sc = wpool.tile([1, N], f32)
            nc.vector.tensor_tensor(sc[:], s1[:], s2[:], Alu.mult)
            nc.sync.dma_start(out[:], sc[:])
            nc.sync.dma_start(dbg_nov[:], nov_r[:])
            nc.sync.dma_start(dbg_uni[:], uni_r[:])
            nc.sync.dma_start(dbg_stab[:], stab[:])

    nc.compile()
    return nc


def _get_nc():
    if "nc" not in _CACHE:
        _CACHE["nc"] = _build()
    return _CACHE["nc"]


def _in_maps(inputs):
    ge = np.ascontiguousarray(inputs["gen_embs_elmd"], dtype=np.float32)
    ga = np.ascontiguousarray(inputs["gen_embs_amd"], dtype=np.float32)
    re = np.ascontiguousarray(inputs["ref_embs_elmd"], dtype=np.float32)
    ra = np.ascontiguousarray(inputs["ref_embs_amd"], dtype=np.float32)
    eah = np.ascontiguousarray(inputs["e_above_hull"], dtype=np.float32).reshape(1, N)
    tri = np.triu(np.ones((DE, DE), dtype=np.float32))
    eye = np.eye(128, dtype=np.float32)
    maps = []
    for c in range(NC):
        maps.append(
            {
                "ge": ge,
                "ga": ga,
                "re": np.ascontiguousarray(re[c * MS : (c + 1) * MS]),
                "ra": np.ascontiguousarray(ra[c * MS : (c + 1) * MS]),
                "mge": np.ascontiguousarray(ge[c * US : (c + 1) * US]),
                "mga": np.ascontiguousarray(ga[c * US : (c + 1) * US]),
                "eah": eah,
                "tri": tri,
                "eye": eye,
            }
        )
    return maps


def kernel(**inputs) -> np.ndarray:
    from concourse.bass_utils import run_bass_kernel_spmd

    nc = _get_nc()
    res = run_bass_kernel_spmd(nc, _in_maps(inputs), list(range(NC)))
    return np.asarray(res.results[0]["scores"], dtype=np.float32).reshape(N)


# revision 24
# speedup vs baseline: 1.2366x; 1.0132x over previous
"""CSUN reward kernel (retrieval_knn) on 8 Trainium2 NeuronCores.

scores[i] = stability(eah[i]) * uni[i] * nov[i]
  d_elmd(i,j) = sum_d |cumsum(a)_id - cumsum(b)_jd|   (L1 on cumsum transform)
  d_amd(i,j)  = max_d |a_id - b_jd|                   (Chebyshev)
  combine(e,c) = Ce*e/(1+e) + Ca*c/(1+c) = (Ce+Ca) - Ce/(1+e) - Ca/(1+c)
  uni[i] = sum_j combine(gen,gen) / (N-1)   nov[i] = min_j combine(gen,ref)

Sharding: ref set M=4096 split 512/core (nov), gen axis split 32/core (uni).
Per core, everything lands in [128 j-partitions, i-free] layout:
  - ELMD: ScalarE activation Abs(Bcs - Acs_i) per i -> [103d, j] abs tile ->
    PE matmul (abs tile as weights, ones as rhs) -> column i of psum [128j, i].
  - AMD: custom DVE op, running Chebyshev max over d on [128j, i-free]:
    acc = max(acc, |arep_d - ra_d|) in one instruction per d.
Combine + min-tree over chunks, PE transpose, free-axis min -> nov partials.
AllGather + strided-reduce combines partials across cores.
"""

import sys

if "/opt/trn_rl_repo" not in sys.path:
    sys.path.insert(0, "/opt/trn_rl_repo")

import numpy as np

N = 256          # gen structures
M = 4096         # ref structures
DE = 103         # elmd dim
DA = 100         # amd dim
NC = 8           # cores
MS = M // NC     # 512 ref per core
US = N // NC     # 32 gen per core (uni shard)
NJT = MS // 128  # 4 ref j-tiles per core

COEF_ELMD = float.fromhex("0x1.8d7d565a99f87p-1")
COEF_AMD = float.fromhex("0x1.ca0aa695981e5p-3")
EAH_SCALE = 0.4289

_CACHE = {}


def _register_dve_op(name, spec, subdim=False, perf_en=None):
    from concourse import dve_ops
    from concourse.dve_spec import lower, _has_src1
    from concourse.dve_uop import DveOpSpec

    for o in dve_ops.OPS:
        if o.name == name:
            return o
    row = max(dve_ops._SUB_OPCODE_FOR_NAME.values()) + 1
    assert row < 0x20
    dve_ops._SUB_OPCODE_FOR_NAME[name] = row
    shas = {}
    for ver in ("v3", "v4"):
        tmp = DveOpSpec(
            name=name, opcode=row, uops=lower(spec, ver=ver), rd1_en=_has_src1(spec)
        )
        shas[ver] = tmp.sha(ver)
    op = dve_ops.DveOp(name, spec, subdim=subdim, uops_sha=shas, perf_en=perf_en or {})
    dve_ops.OPS.append(op)
    dve_ops.CUSTOM_DVE_SPECS[name] = spec
    return op


def _get_ops():
    from concourse.dve_spec import Spec, Src0, Src1, C0, C1, maxx

    absdiff = _register_dve_op(
        "ABSDIFF_ANT",
        Spec(
            body=maxx(Src0 - C0, C0 - Src0),
            reference=lambda in0, in1, s0, s1, imm2: np.maximum(
                in0.astype(np.float32) - s0, s0 - in0.astype(np.float32)
            ).astype(np.float32),
        ),
    )
    pairmax = _register_dve_op(
        "PAIRMAX_ANT",
        Spec(
            body=maxx(maxx(Src0 - C0, C0 - Src0), maxx(Src1 - C1, C1 - Src1)),
            reference=lambda in0, in1, s0, s1, imm2: np.maximum(
                np.maximum(
                    in0.astype(np.float32) - s0, s0 - in0.astype(np.float32)
                ),
                np.maximum(
                    in1.astype(np.float32) - s1, s1 - in1.astype(np.float32)
                ),
            ).astype(np.float32),
        ),
    )
    chebacc = _register_dve_op(
        "CHEBACC_ANT",
        Spec(
            body=maxx(maxx(Src0 - C0, C0 - Src0), Src1),
            reference=lambda in0, in1, s0, s1, imm2: np.maximum(
                np.maximum(in0.astype(np.float32) - s0, s0 - in0.astype(np.float32)),
                in1.astype(np.float32),
            ).astype(np.float32),
        ),
    )
    return absdiff, chebacc, pairmax


def _build():
    from concourse import bacc, tile, mybir

    f32 = mybir.dt.float32
    bf16 = mybir.dt.float16  # fp16: same DVE perf modes, 8x finer ulp here
    Alu = mybir.AluOpType
    AFT = mybir.ActivationFunctionType
    X = mybir.AxisListType.X

    ABSDIFF, CHEBACC, PAIRMAX = _get_ops()

    nc = bacc.Bacc("TRN2", target_bir_lowering=False, debug=False, num_devices=NC)

    ge = nc.dram_tensor("ge", [N, DE], f32, kind="ExternalInput").ap()
    ga = nc.dram_tensor("ga", [N, DA], f32, kind="ExternalInput").ap()
    re = nc.dram_tensor("re", [MS, DE], f32, kind="ExternalInput").ap()
    ra = nc.dram_tensor("ra", [MS, DA], f32, kind="ExternalInput").ap()
    mge = nc.dram_tensor("mge", [US, DE], f32, kind="ExternalInput").ap()
    mga = nc.dram_tensor("mga", [US, DA], f32, kind="ExternalInput").ap()
    eah = nc.dram_tensor("eah", [1, N], f32, kind="ExternalInput").ap()
    tri = nc.dram_tensor("tri", [DE, DE], f32, kind="ExternalInput").ap()
    eye = nc.dram_tensor("eye", [128, 128], f32, kind="ExternalInput").ap()
    out = nc.dram_tensor("scores", [1, N], f32, kind="ExternalOutput").ap()
    dbg_nov = nc.dram_tensor("dbg_nov", [1, N], f32, kind="ExternalOutput").ap()
    dbg_uni = nc.dram_tensor("dbg_uni", [1, N], f32, kind="ExternalOutput").ap()
    dbg_stab = nc.dram_tensor("dbg_stab", [1, N], f32, kind="ExternalOutput").ap()

    CE, CA = COEF_ELMD, COEF_AMD

    with tile.TileContext(nc) as tc:
        with (
            tc.tile_pool(name="const", bufs=1) as cpool,
            tc.tile_pool(name="work", bufs=1) as wpool,
            tc.tile_pool(name="abs", bufs=4) as apool,
            tc.tile_pool(name="acc", bufs=2) as accpool,
            tc.tile_pool(name="pm", bufs=8) as pmpool,
            tc.tile_pool(name="comb", bufs=1) as combpool,
            tc.tile_pool(name="psA", bufs=4, space="PSUM") as psA,
            tc.tile_pool(name="psB", bufs=2, space="PSUM") as psB,
            tc.tile_pool(name="psC", bufs=2, space="PSUM") as psC,
            tc.tile_pool(name="dram", bufs=1, space="DRAM") as dpool,
        ):
            # ---------------- prep ----------------
            tri_s = cpool.tile([DE, DE], f32)
            nc.sync.dma_start(tri_s[:], tri[:])
            eye_s = cpool.tile([128, 128], f32)
            nc.sync.dma_start(eye_s[:], eye[:])

            # per-partition scalar sources (also reused as transpose sources)
            ra_f = []
            for jt in range(NJT):
                t_f = cpool.tile([128, DA], f32, tag=f"raf{jt}", name=f"raf{jt}")
                nc.sync.dma_start(t_f[:], ra[jt * 128 : (jt + 1) * 128, :])
                ra_f.append(t_f)
            ga_s = []
            for ch in range(2):
                t_f = cpool.tile([128, DA], f32, tag=f"gas{ch}", name=f"gas{ch}")
                nc.sync.dma_start(t_f[:], ga[ch * 128 : (ch + 1) * 128, :])
                ga_s.append(t_f)

            # raw row-major loads used only as PE-transpose sources
            ge_r = []
            for ch in range(2):
                t = wpool.tile([128, DE], f32, tag=f"ger{ch}", name=f"ger{ch}")
                nc.sync.dma_start(t[:], ge[ch * 128 : (ch + 1) * 128, :])
                ge_r.append(t)
            re_r = []
            for jt in range(NJT):
                t = wpool.tile([128, DE], f32, tag=f"rer{jt}", name=f"rer{jt}")
                nc.sync.dma_start(t[:], re[jt * 128 : (jt + 1) * 128, :])
                re_r.append(t)
            mge_r = wpool.tile([US, DE], f32)
            nc.sync.dma_start(mge_r[:], mge[:])
            mga_r = wpool.tile([US, DA], f32)
            nc.sync.dma_start(mga_r[:], mga[:])

            # transpose via PE (x.T @ I), evacuate PSUM on ScalarE
            geT = wpool.tile([DE, N], f32)
            reT = wpool.tile([DE, MS], f32)
            gaT = wpool.tile([DA, N], f32)
            mgeT = wpool.tile([DE, US], f32)
            mgaT = wpool.tile([DA, US], f32)
            prep_tr = [
                (ge_r[0], geT[:, 0:128]),
                (ge_r[1], geT[:, 128:256]),
                (re_r[0], reT[:, 0:128]),
                (re_r[1], reT[:, 128:256]),
                (re_r[2], reT[:, 256:384]),
                (re_r[3], reT[:, 384:512]),
                (ga_s[0], gaT[:, 0:128]),
                (ga_s[1], gaT[:, 128:256]),
            ]
            for k, (src, dst) in enumerate(prep_tr):
                dcols = src.shape[1]
                tr = psB.tile([dcols, 128], f32, tag="tr", name=f"ptr{k}")
                nc.tensor.transpose(tr[:], src[:], eye_s[:])
                nc.scalar.copy(out=dst, in_=tr[:])
            tr_m = psB.tile([DE, US], f32, tag="tr", name="ptr_mge")
            nc.tensor.transpose(tr_m[:], mge_r[:], eye_s[0:US, 0:US])
            nc.scalar.copy(out=mgeT[:], in_=tr_m[:])
            tr_g = psB.tile([DA, US], f32, tag="tr", name="ptr_mga")
            nc.tensor.transpose(tr_g[:], mga_r[:], eye_s[0:US, 0:US])
            nc.scalar.copy(out=mgaT[:], in_=tr_g[:])

            # cumsum along d via upper-tri matmul: cs^T[d,i] = sum_{e<=d} x^T[e,i]
            csA_ps = psA.tile([DE, N], f32, tag="big")
            nc.tensor.matmul(csA_ps[:], tri_s[:], geT[:], start=True, stop=True)
            csB_ps = psA.tile([DE, MS], f32, tag="big")
            nc.tensor.matmul(csB_ps[:], tri_s[:], reT[:], start=True, stop=True)
            csM_ps = psA.tile([DE, US], f32, tag="big")
            nc.tensor.matmul(csM_ps[:], tri_s[:], mgeT[:], start=True, stop=True)

            Acs = cpool.tile([DE, N], bf16)
            nc.vector.tensor_copy(Acs[:], csA_ps[:])
            AcsN = cpool.tile([DE, N], bf16)  # negated fp16 cumsum, ACT bias
            nc.vector.tensor_scalar(
                out=AcsN[:], in0=Acs[:], scalar1=-1.0, scalar2=None, op0=Alu.mult
            )
            Bcs = cpool.tile([DE, MS], bf16)
            nc.vector.tensor_copy(Bcs[:], csB_ps[:])
            # round through bf16 so |Acs_i - Mcs_j| is exactly 0 on the uni
            # diagonal (saturating e/(1+e) otherwise turns rounding noise
            # into a large spurious diagonal term)
            McsH = cpool.tile([DE, US], bf16)
            nc.vector.tensor_copy(McsH[:], csM_ps[:])
            McsN = cpool.tile([DE, US], bf16)
            nc.vector.tensor_scalar(
                out=McsN[:], in0=McsH[:], scalar1=-1.0, scalar2=None, op0=Alu.mult
            )

            # replicated gen_amd^T, d-major: arep[p, d*N+i] = ga[i, d]
            gaT_bf = wpool.tile([DA, N], bf16)
            nc.vector.tensor_copy(gaT_bf[:], gaT[:])
            flat_d = dpool.tile([1, DA * N], bf16)
            nc.sync.dma_start(flat_d[:], gaT_bf[:])
            arep = cpool.tile([128, DA * N], bf16)
            DCH = 20  # d's per chunk: replicate via 0-stride DMA reads
            for c0 in range(0, DA, DCH):
                nc.sync.dma_start(
                    arep[:, c0 * N : (c0 + DCH) * N],
                    flat_d[:, c0 * N : (c0 + DCH) * N].partition_broadcast(128),
                )

            # replicated my-gen amd^T for uni: mrep[p, d*US+j] = mga[j, d]
            mgaT_bf = wpool.tile([DA, US], bf16)
            nc.vector.tensor_copy(mgaT_bf[:], mgaT[:])
            mflat_d = dpool.tile([1, DA * US], bf16)
            nc.sync.dma_start(mflat_d[:], mgaT_bf[:])
            mrep = cpool.tile([128, DA * US], bf16)
            nc.sync.dma_start(mrep[:], mflat_d.partition_broadcast(128))

            eah_s = cpool.tile([1, N], f32)
            nc.sync.dma_start(eah_s[:], eah[:])

            ones_e = cpool.tile([DE, 1], bf16)
            nc.vector.memset(ones_e[:], 1.0)

            def cheb_tree(pool, width, in_of, s_of, tag):
                # pairwise |a-b| max over d via PAIRMAX + streaming binary
                # merge tree of fp16 tensor_tensor(max); max ~log2 live tiles
                stack = []  # (level, tile)
                for k in range(DA // 2):
                    t = pool.tile([128, width], bf16, tag=tag, name=f"{tag}_{k}")
                    nc.vector._custom_dve(
                        PAIRMAX,
                        out=t[:],
                        in0=in_of(2 * k),
                        in1=in_of(2 * k + 1),
                        s0=s_of(2 * k),
                        s1=s_of(2 * k + 1),
                    )
                    lvl = 0
                    while stack and stack[-1][0] == lvl:
                        _, prev = stack.pop()
                        nc.vector.tensor_tensor(prev[:], prev[:], t[:], Alu.max)
                        t = prev
                        lvl += 1
                    stack.append((lvl, t))
                while len(stack) > 1:
                    _, a = stack.pop()
                    _, b = stack.pop()
                    nc.vector.tensor_tensor(b[:], b[:], a[:], Alu.max)
                    stack.append((99, b))
                return stack[0][1]

            uET_ps = [psC.tile([128, US], f32, tag="u", name=f"uET{k}") for k in range(2)]
            for j in range(US):
                ab = apool.tile([DE, N], bf16, tag="uab")
                nc.scalar.activation(
                    out=ab[:],
                    in_=Acs[:],
                    func=AFT.Abs,
                    bias=McsN[:, j : j + 1],
                    scale=1.0,
                )
                for ch in range(2):
                    nc.tensor.matmul(
                        uET_ps[ch][:, j : j + 1],
                        ab[:, ch * 128 : (ch + 1) * 128],
                        ones_e[:],
                        start=True,
                        stop=True,
                    )
            ustack = []
            for k in range(DA // 2):
                t = pmpool.tile([128, 2 * US], bf16, tag="upmS", name=f"upmS_{k}")
                for ch in range(2):
                    nc.vector._custom_dve(
                        PAIRMAX,
                        out=t[:, ch * US : (ch + 1) * US],
                        in0=mrep[:, 2 * k * US : (2 * k + 1) * US],
                        in1=mrep[:, (2 * k + 1) * US : (2 * k + 2) * US],
                        s0=ga_s[ch][:, 2 * k : 2 * k + 1],
                        s1=ga_s[ch][:, 2 * k + 1 : 2 * k + 2],
                    )
                lvl = 0
                while ustack and ustack[-1][0] == lvl:
                    _, prev = ustack.pop()
                    nc.vector.tensor_tensor(prev[:], prev[:], t[:], Alu.max)
                    t = prev
                    lvl += 1
                ustack.append((lvl, t))
            while len(ustack) > 1:
                _, a = ustack.pop()
                _, b = ustack.pop()
                nc.vector.tensor_tensor(b[:], b[:], a[:], Alu.max)
                ustack.append((99, b))
            uacc_all = ustack[0][1]
            uni_p = []
            for ch in range(2):
                uacc = uacc_all[:, ch * US : (ch + 1) * US]
                uc1 = wpool.tile([128, US], f32, tag=f"uc1{ch}", name=f"uc1{ch}")
                nc.vector.tensor_scalar(
                    out=uc1[:], in0=uacc[:], scalar1=1.0, scalar2=None, op0=Alu.add
                )
                ur_a = wpool.tile([128, US], f32, tag=f"ura{ch}", name=f"ura{ch}")
                ursc = wpool.tile([128, US], f32, tag=f"ursc{ch}", name=f"ursc{ch}")
                nc.vector.reciprocal_approx_accurate(out=ur_a[:], in_=uc1[:], scratch=ursc[:])
                ux = wpool.tile([128, US], f32, tag=f"ux{ch}", name=f"ux{ch}")
                nc.vector.tensor_scalar(
                    out=ux[:],
                    in0=ur_a[:],
                    scalar1=-CA,
                    scalar2=CE + CA,
                    op0=Alu.mult,
                    op1=Alu.add,
                )
                ue1 = wpool.tile([128, US], f32, tag=f"ue1{ch}", name=f"ue1{ch}")
                nc.scalar.activation(
                    out=ue1[:], in_=uET_ps[ch][:], func=AFT.Copy, bias=1.0, scale=1.0
                )
                ur_e = wpool.tile([128, US], f32, tag=f"ure{ch}", name=f"ure{ch}")
                ursc2 = wpool.tile([128, US], f32, tag=f"ursc2{ch}", name=f"ursc2{ch}")
                nc.vector.reciprocal_approx_accurate(out=ur_e[:], in_=ue1[:], scratch=ursc2[:])
                ucomb = wpool.tile([128, US], f32, tag=f"ucomb{ch}", name=f"ucomb{ch}")
                nc.vector.scalar_tensor_tensor(
                    out=ucomb[:],
                    in0=ur_e[:],
                    scalar=-CE,
                    in1=ux[:],
                    op0=Alu.mult,
                    op1=Alu.add,
                )
                up = wpool.tile([128, 1], f32, tag=f"up{ch}", name=f"up{ch}")
                nc.vector.tensor_reduce(out=up[:], in_=ucomb[:], axis=X, op=Alu.add)
                uni_p.append(up)

            cin_u = dpool.tile([1, N], f32)
            nc.sync.dma_start(cin_u[:, 0:128], uni_p[0][:])
            nc.sync.dma_start(cin_u[:, 128:256], uni_p[1][:])
            cout_u = dpool.tile([NC, N], f32)
            nc.gpsimd.collective_compute(
                "AllGather",
                Alu.bypass,
                replica_groups=[list(range(NC))],
                ins=[cin_u.opt()],
                outs=[cout_u.opt()],
            )
            uni_cat = wpool.tile([1, NC * N], f32)
            nc.sync.dma_start(uni_cat[:], cout_u[:])
            uni_r = wpool.tile([1, N], f32)
            nc.vector.tensor_reduce(
                out=uni_r[:],
                in_=uni_cat.rearrange("p (c n) -> p n c", c=NC),
                axis=X,
                op=Alu.add,
            )

            # ---- ELMD nov via min identity: |a-b| = a+b-2*min(a,b) ------
            # ET[jt][j, i] = sum_d min(A_id, B_jd); SAB[jt][j, i] = 1+SA_i+SB_j
            # row/col sums SA [1,N], SB [1,MS] via PE ones-matmuls
            ET_ps = [
                psA.tile([128, N], f32, tag="big", name=f"ET{k}") for k in range(NJT)
            ]
            for i in range(N):
                ab = apool.tile([DE, MS], bf16, tag="ab")
                nc.scalar.activation(
                    out=ab[:],
                    in_=Bcs[:],
                    func=AFT.Abs,
                    bias=AcsN[:, i : i + 1],
                    scale=1.0,
                )
                for jt in range(NJT):
                    nc.tensor.matmul(
                        ET_ps[jt][:, i : i + 1],
                        ab[:, jt * 128 : (jt + 1) * 128],
                        ones_e[:],
                        start=True,
                        stop=True,
                    )

            # ---- AMD nov + combine per j-tile -> comb[jt] [128j, 256i] ----
            # all 4 j-tiles share one wide tile per pair so the merge tree
            # runs 4x-wide ops (fewer dispatches)
            stack = []
            for k in range(DA // 2):
                t = pmpool.tile([128, NJT * N], bf16, tag="pmS", name=f"pmS_{k}")
                for jt in range(NJT):
                    nc.vector._custom_dve(
                        PAIRMAX,
                        out=t[:, jt * N : (jt + 1) * N],
                        in0=arep[:, 2 * k * N : (2 * k + 1) * N],
                        in1=arep[:, (2 * k + 1) * N : (2 * k + 2) * N],
                        s0=ra_f[jt][:, 2 * k : 2 * k + 1],
                        s1=ra_f[jt][:, 2 * k + 1 : 2 * k + 2],
                    )
                lvl = 0
                while stack and stack[-1][0] == lvl:
                    _, prev = stack.pop()
                    nc.vector.tensor_tensor(prev[:], prev[:], t[:], Alu.max)
                    t = prev
                    lvl += 1
                stack.append((lvl, t))
            while len(stack) > 1:
                _, a = stack.pop()
                _, b = stack.pop()
                nc.vector.tensor_tensor(b[:], b[:], a[:], Alu.max)
                stack.append((99, b))
            acc_all = stack[0][1]
            accs = [acc_all[:, jt * N : (jt + 1) * N] for jt in range(NJT)]
            comb = []
            for jt in range(NJT):
                # affine stages on ScalarE (ACT), reciprocals on DVE
                c1f = accpool.tile([128, N], f32, tag="c1f")
                nc.scalar.activation(
                    out=c1f[:], in_=accs[jt][:], func=AFT.Copy, bias=1.0, scale=1.0
                )
                r_a = accpool.tile([128, N], f32, tag="r_a")
                rsc = accpool.tile([128, N], f32, tag="rsc")
                nc.vector.reciprocal_approx_accurate(out=r_a[:], in_=c1f[:], scratch=rsc[:])
                # x = (CE+CA) - CA*r_a
                x = accpool.tile([128, N], f32, tag="x")
                nc.scalar.activation(
                    out=x[:], in_=r_a[:], func=AFT.Copy, bias=CE + CA, scale=-CA
                )
                e1 = accpool.tile([128, N], f32, tag="e1")
                nc.scalar.activation(
                    out=e1[:], in_=ET_ps[jt][:], func=AFT.Copy, bias=1.0, scale=1.0
                )
                r_e = accpool.tile([128, N], f32, tag="r_e")
                rsc2 = accpool.tile([128, N], f32, tag="rsc2")
                nc.vector.reciprocal_approx_accurate(out=r_e[:], in_=e1[:], scratch=rsc2[:])
                cb = combpool.tile([128, N], f32, tag=f"comb{jt}", name=f"comb{jt}")
                nc.vector.scalar_tensor_tensor(
                    out=cb[:],
                    in0=r_e[:],
                    scalar=-CE,
                    in1=x[:],
                    op0=Alu.mult,
                    op1=Alu.add,
                )
                comb.append(cb)

            # min over the 4 chunks, then transpose + free-axis min over j
            nc.vector.tensor_tensor(comb[0][:], comb[0][:], comb[1][:], Alu.min)
            nc.vector.tensor_tensor(comb[2][:], comb[2][:], comb[3][:], Alu.min)
            nc.vector.tensor_tensor(comb[0][:], comb[0][:], comb[2][:], Alu.min)
            nov_p = []
            for h in range(2):
                tr = psB.tile([128, 128], f32, tag="tr", name=f"tr{h}")
                nc.tensor.transpose(tr[:], comb[0][:, h * 128 : (h + 1) * 128], eye_s[:])
                np_t = combpool.tile([128, 1], f32, tag=f"novp{h}", name=f"novp{h}")
                nc.vector.tensor_reduce(out=np_t[:], in_=tr[:], axis=X, op=Alu.min)
                nov_p.append(np_t)

            # ---- uni (gen x gen, my 32 j columns): [128i, 32j] layout -----
            # ---------------- nov allgather (tail) --------------------------
            cin = dpool.tile([1, N], f32)
            nc.sync.dma_start(cin[:, 0:128], nov_p[0][:])
            nc.sync.dma_start(cin[:, 128:256], nov_p[1][:])
            cout = dpool.tile([NC, N], f32)
            nc.gpsimd.collective_compute(
                "AllGather",
                Alu.bypass,
                replica_groups=[list(range(NC))],
                ins=[cin.opt()],
                outs=[cout.opt()],
            )
            nov_cat = wpool.tile([1, NC * N], f32)
            nc.sync.dma_start(nov_cat[:], cout[:])
            nov_r = wpool.tile([1, N], f32)
            nc.vector.tensor_reduce(
                out=nov_r[:],
                in_=nov_cat.rearrange("p (c n) -> p n c", c=NC),
                axis=X,
                op=Alu.min,
            )

            # ---------------- final ----------------------------------------
            stab = wpool.tile([1, N], f32)
            nc.vector.tensor_scalar(
                out=stab[:],
                in0=eah_s[:],
                scalar1=-1.0 / EAH_SCALE,
                scalar2=1.0,
                op0=Alu.mult,
                op1=Alu.add,
            )
            nc.vector.tensor_scalar(
                out=stab[:],
                in0=stab[:],
                scalar1=0.0,
                scalar2=1.0,
                op0=Alu.max,
                op1=Alu.min,
            )
            s1 = wpool.tile([1, N], f32)
            nc.vector.tensor_tensor(s1[:], stab[:], nov_r[:], Alu.mult)
            s2 = wpool.tile([1, N], f32)
            nc.vector.tensor_scalar(
                out=s2[:],
                in0=uni_r[:],
                scalar1=1.0 / (N - 1),
                scalar2=None,
                op0=Alu.mult,
            )
            sc = wpool.tile([1, N], f32)
            nc.vector.tensor_tensor(sc[:], s1[:], s2[:], Alu.mult)
            nc.sync.dma_start(out[:], sc[:])
            nc.sync.dma_start(dbg_nov[:], nov_r[:])
            nc.sync.dma_start(dbg_uni[:], uni_r[:])
            nc.sync.dma_start(dbg_stab[:], stab[:])

    nc.compile()
    return nc


def _get_nc():
    if "nc" not in _CACHE:
        _CACHE["nc"] = _build()
    return _CACHE["nc"]


def _in_maps(inputs):
    ge = np.ascontiguousarray(inputs["gen_embs_elmd"], dtype=np.float32)
    ga = np.ascontiguousarray(inputs["gen_embs_amd"], dtype=np.float32)
    re = np.ascontiguousarray(inputs["ref_embs_elmd"], dtype=np.float32)
    ra = np.ascontiguousarray(inputs["ref_embs_amd"], dtype=np.float32)
    eah = np.ascontiguousarray(inputs["e_above_hull"], dtype=np.float32).reshape(1, N)
    tri = np.triu(np.ones((DE, DE), dtype=np.float32))
    eye = np.eye(128, dtype=np.float32)
    maps = []
    for c in range(NC):
        maps.append(
            {
                "ge": ge,
                "ga": ga,
                "re": np.ascontiguousarray(re[c * MS : (c + 1) * MS]),
                "ra": np.ascontiguousarray(ra[c * MS : (c + 1) * MS]),
                "mge": np.ascontiguousarray(ge[c * US : (c + 1) * US]),
                "mga": np.ascontiguousarray(ga[c * US : (c + 1) * US]),
                "eah": eah,
                "tri": tri,
                "eye": eye,
            }
        )
    return maps


def kernel(**inputs) -> np.ndarray:
    from concourse.bass_utils import run_bass_kernel_spmd

    nc = _get_nc()
    res = run_bass_kernel_spmd(nc, _in_maps(inputs), list(range(NC)))
    return np.asarray(res.results[0]["scores"], dtype=np.float32).reshape(N)


# revision 25
# speedup vs baseline: 1.2557x; 1.0154x over previous
"""CSUN reward kernel (retrieval_knn) on 8 Trainium2 NeuronCores.

scores[i] = stability(eah[i]) * uni[i] * nov[i]
  d_elmd(i,j) = sum_d |cumsum(a)_id - cumsum(b)_jd|   (L1 on cumsum transform)
  d_amd(i,j)  = max_d |a_id - b_jd|                   (Chebyshev)
  combine(e,c) = Ce*e/(1+e) + Ca*c/(1+c) = (Ce+Ca) - Ce/(1+e) - Ca/(1+c)
  uni[i] = sum_j combine(gen,gen) / (N-1)   nov[i] = min_j combine(gen,ref)

Sharding: ref set M=4096 split 512/core (nov), gen axis split 32/core (uni).
Per core, everything lands in [128 j-partitions, i-free] layout:
  - ELMD: ScalarE activation Abs(Bcs - Acs_i) per i -> [103d, j] abs tile ->
    PE matmul (abs tile as weights, ones as rhs) -> column i of psum [128j, i].
  - AMD: custom DVE op, running Chebyshev max over d on [128j, i-free]:
    acc = max(acc, |arep_d - ra_d|) in one instruction per d.
Combine + min-tree over chunks, PE transpose, free-axis min -> nov partials.
AllGather + strided-reduce combines partials across cores.
"""

import sys

if "/opt/trn_rl_repo" not in sys.path:
    sys.path.insert(0, "/opt/trn_rl_repo")

import numpy as np

N = 256          # gen structures
M = 4096         # ref structures
DE = 103         # elmd dim
DA = 100         # amd dim
NC = 8           # cores
MS = M // NC     # 512 ref per core
US = N // NC     # 32 gen per core (uni shard)
NJT = MS // 128  # 4 ref j-tiles per core

COEF_ELMD = float.fromhex("0x1.8d7d565a99f87p-1")
COEF_AMD = float.fromhex("0x1.ca0aa695981e5p-3")
EAH_SCALE = 0.4289

_CACHE = {}


def _register_dve_op(name, spec, subdim=False, perf_en=None):
    from concourse import dve_ops
    from concourse.dve_spec import lower, _has_src1
    from concourse.dve_uop import DveOpSpec

    for o in dve_ops.OPS:
        if o.name == name:
            return o
    row = max(dve_ops._SUB_OPCODE_FOR_NAME.values()) + 1
    assert row < 0x20
    dve_ops._SUB_OPCODE_FOR_NAME[name] = row
    shas = {}
    for ver in ("v3", "v4"):
        tmp = DveOpSpec(
            name=name, opcode=row, uops=lower(spec, ver=ver), rd1_en=_has_src1(spec)
        )
        shas[ver] = tmp.sha(ver)
    op = dve_ops.DveOp(name, spec, subdim=subdim, uops_sha=shas, perf_en=perf_en or {})
    dve_ops.OPS.append(op)
    dve_ops.CUSTOM_DVE_SPECS[name] = spec
    return op


def _get_ops():
    from concourse.dve_spec import Spec, Src0, Src1, C0, C1, maxx

    absdiff = _register_dve_op(
        "ABSDIFF_ANT",
        Spec(
            body=maxx(Src0 - C0, C0 - Src0),
            reference=lambda in0, in1, s0, s1, imm2: np.maximum(
                in0.astype(np.float32) - s0, s0 - in0.astype(np.float32)
            ).astype(np.float32),
        ),
    )
    pairmax = _register_dve_op(
        "PAIRMAX_ANT",
        Spec(
            body=maxx(maxx(Src0 - C0, C0 - Src0), maxx(Src1 - C1, C1 - Src1)),
            reference=lambda in0, in1, s0, s1, imm2: np.maximum(
                np.maximum(
                    in0.astype(np.float32) - s0, s0 - in0.astype(np.float32)
                ),
                np.maximum(
                    in1.astype(np.float32) - s1, s1 - in1.astype(np.float32)
                ),
            ).astype(np.float32),
        ),
    )
    chebacc = _register_dve_op(
        "CHEBACC_ANT",
        Spec(
            body=maxx(maxx(Src0 - C0, C0 - Src0), Src1),
            reference=lambda in0, in1, s0, s1, imm2: np.maximum(
                np.maximum(in0.astype(np.float32) - s0, s0 - in0.astype(np.float32)),
                in1.astype(np.float32),
            ).astype(np.float32),
        ),
    )
    return absdiff, chebacc, pairmax


def _build():
    from concourse import bacc, tile, mybir

    f32 = mybir.dt.float32
    bf16 = mybir.dt.float16  # fp16: same DVE perf modes, 8x finer ulp here
    Alu = mybir.AluOpType
    AFT = mybir.ActivationFunctionType
    X = mybir.AxisListType.X

    ABSDIFF, CHEBACC, PAIRMAX = _get_ops()

    nc = bacc.Bacc("TRN2", target_bir_lowering=False, debug=False, num_devices=NC)

    ge = nc.dram_tensor("ge", [N, DE], f32, kind="ExternalInput").ap()
    ga = nc.dram_tensor("ga", [N, DA], f32, kind="ExternalInput").ap()
    re = nc.dram_tensor("re", [MS, DE], f32, kind="ExternalInput").ap()
    ra = nc.dram_tensor("ra", [MS, DA], f32, kind="ExternalInput").ap()
    mge = nc.dram_tensor("mge", [US, DE], f32, kind="ExternalInput").ap()
    mga = nc.dram_tensor("mga", [US, DA], f32, kind="ExternalInput").ap()
    eah = nc.dram_tensor("eah", [1, N], f32, kind="ExternalInput").ap()
    tri = nc.dram_tensor("tri", [DE, DE], f32, kind="ExternalInput").ap()
    eye = nc.dram_tensor("eye", [128, 128], f32, kind="ExternalInput").ap()
    out = nc.dram_tensor("scores", [1, N], f32, kind="ExternalOutput").ap()
    dbg_nov = nc.dram_tensor("dbg_nov", [1, N], f32, kind="ExternalOutput").ap()
    dbg_uni = nc.dram_tensor("dbg_uni", [1, N], f32, kind="ExternalOutput").ap()
    dbg_stab = nc.dram_tensor("dbg_stab", [1, N], f32, kind="ExternalOutput").ap()

    CE, CA = COEF_ELMD, COEF_AMD

    with tile.TileContext(nc) as tc:
        with (
            tc.tile_pool(name="const", bufs=1) as cpool,
            tc.tile_pool(name="work", bufs=1) as wpool,
            tc.tile_pool(name="abs", bufs=4) as apool,
            tc.tile_pool(name="acc", bufs=2) as accpool,
            tc.tile_pool(name="pm", bufs=8) as pmpool,
            tc.tile_pool(name="comb", bufs=1) as combpool,
            tc.tile_pool(name="psA", bufs=4, space="PSUM") as psA,
            tc.tile_pool(name="psB", bufs=2, space="PSUM") as psB,
            tc.tile_pool(name="psC", bufs=2, space="PSUM") as psC,
            tc.tile_pool(name="dram", bufs=1, space="DRAM") as dpool,
        ):
            # ---------------- prep ----------------
            tri_s = cpool.tile([DE, DE], f32)
            nc.sync.dma_start(tri_s[:], tri[:])
            eye_s = cpool.tile([128, 128], f32)
            nc.sync.dma_start(eye_s[:], eye[:])

            # per-partition scalar sources (also reused as transpose sources)
            ra_f = []
            for jt in range(NJT):
                t_f = cpool.tile([128, DA], f32, tag=f"raf{jt}", name=f"raf{jt}")
                nc.sync.dma_start(t_f[:], ra[jt * 128 : (jt + 1) * 128, :])
                ra_f.append(t_f)
            ga_s = []
            for ch in range(2):
                t_f = cpool.tile([128, DA], f32, tag=f"gas{ch}", name=f"gas{ch}")
                nc.sync.dma_start(t_f[:], ga[ch * 128 : (ch + 1) * 128, :])
                ga_s.append(t_f)

            # raw row-major loads used only as PE-transpose sources
            ge_r = []
            for ch in range(2):
                t = wpool.tile([128, DE], f32, tag=f"ger{ch}", name=f"ger{ch}")
                nc.sync.dma_start(t[:], ge[ch * 128 : (ch + 1) * 128, :])
                ge_r.append(t)
            re_r = []
            for jt in range(NJT):
                t = wpool.tile([128, DE], f32, tag=f"rer{jt}", name=f"rer{jt}")
                nc.sync.dma_start(t[:], re[jt * 128 : (jt + 1) * 128, :])
                re_r.append(t)
            mge_r = wpool.tile([US, DE], f32)
            nc.sync.dma_start(mge_r[:], mge[:])
            mga_r = wpool.tile([US, DA], f32)
            nc.sync.dma_start(mga_r[:], mga[:])

            # transpose via PE (x.T @ I), evacuate PSUM on ScalarE
            geT = wpool.tile([DE, N], f32)
            reT = wpool.tile([DE, MS], f32)
            gaT = wpool.tile([DA, N], f32)
            mgeT = wpool.tile([DE, US], f32)
            mgaT = wpool.tile([DA, US], f32)
            prep_tr = [
                (ge_r[0], geT[:, 0:128]),
                (ge_r[1], geT[:, 128:256]),
                (re_r[0], reT[:, 0:128]),
                (re_r[1], reT[:, 128:256]),
                (re_r[2], reT[:, 256:384]),
                (re_r[3], reT[:, 384:512]),
                (ga_s[0], gaT[:, 0:128]),
                (ga_s[1], gaT[:, 128:256]),
            ]
            for k, (src, dst) in enumerate(prep_tr):
                dcols = src.shape[1]
                tr = psB.tile([dcols, 128], f32, tag="tr", name=f"ptr{k}")
                nc.tensor.transpose(tr[:], src[:], eye_s[:])
                nc.scalar.copy(out=dst, in_=tr[:])
            tr_m = psB.tile([DE, US], f32, tag="tr", name="ptr_mge")
            nc.tensor.transpose(tr_m[:], mge_r[:], eye_s[0:US, 0:US])
            nc.scalar.copy(out=mgeT[:], in_=tr_m[:])
            tr_g = psB.tile([DA, US], f32, tag="tr", name="ptr_mga")
            nc.tensor.transpose(tr_g[:], mga_r[:], eye_s[0:US, 0:US])
            nc.scalar.copy(out=mgaT[:], in_=tr_g[:])

            # cumsum along d via upper-tri matmul: cs^T[d,i] = sum_{e<=d} x^T[e,i]
            csA_ps = psA.tile([DE, N], f32, tag="big")
            nc.tensor.matmul(csA_ps[:], tri_s[:], geT[:], start=True, stop=True)
            csB_ps = psA.tile([DE, MS], f32, tag="big")
            nc.tensor.matmul(csB_ps[:], tri_s[:], reT[:], start=True, stop=True)
            csM_ps = psA.tile([DE, US], f32, tag="big")
            nc.tensor.matmul(csM_ps[:], tri_s[:], mgeT[:], start=True, stop=True)

            Acs = cpool.tile([DE, N], bf16)
            nc.vector.tensor_copy(Acs[:], csA_ps[:])
            AcsN = cpool.tile([DE, N], bf16)  # negated fp16 cumsum, ACT bias
            nc.vector.tensor_scalar(
                out=AcsN[:], in0=Acs[:], scalar1=-1.0, scalar2=None, op0=Alu.mult
            )
            Bcs = cpool.tile([DE, MS], bf16)
            nc.vector.tensor_copy(Bcs[:], csB_ps[:])
            # round through bf16 so |Acs_i - Mcs_j| is exactly 0 on the uni
            # diagonal (saturating e/(1+e) otherwise turns rounding noise
            # into a large spurious diagonal term)
            McsH = cpool.tile([DE, US], bf16)
            nc.vector.tensor_copy(McsH[:], csM_ps[:])
            McsN = cpool.tile([DE, US], bf16)
            nc.vector.tensor_scalar(
                out=McsN[:], in0=McsH[:], scalar1=-1.0, scalar2=None, op0=Alu.mult
            )

            # replicated gen_amd^T, d-major: arep[p, d*N+i] = ga[i, d]
            gaT_bf = wpool.tile([DA, N], bf16)
            nc.vector.tensor_copy(gaT_bf[:], gaT[:])
            flat_d = dpool.tile([1, DA * N], bf16)
            nc.sync.dma_start(flat_d[:], gaT_bf[:])
            arep = cpool.tile([128, DA * N], bf16)
            DCH = 20  # d's per chunk: replicate via 0-stride DMA reads
            for c0 in range(0, DA, DCH):
                nc.sync.dma_start(
                    arep[:, c0 * N : (c0 + DCH) * N],
                    flat_d[:, c0 * N : (c0 + DCH) * N].partition_broadcast(128),
                )

            # replicated my-gen amd^T for uni: mrep[p, d*US+j] = mga[j, d]
            mgaT_bf = wpool.tile([DA, US], bf16)
            nc.vector.tensor_copy(mgaT_bf[:], mgaT[:])
            mflat_d = dpool.tile([1, DA * US], bf16)
            nc.sync.dma_start(mflat_d[:], mgaT_bf[:])
            mrep = cpool.tile([128, DA * US], bf16)
            nc.sync.dma_start(mrep[:], mflat_d.partition_broadcast(128))

            eah_s = cpool.tile([1, N], f32)
            nc.sync.dma_start(eah_s[:], eah[:])

            ones_e = cpool.tile([DE, 1], bf16)
            nc.vector.memset(ones_e[:], 1.0)

            def cheb_tree(pool, width, in_of, s_of, tag):
                # pairwise |a-b| max over d via PAIRMAX + streaming binary
                # merge tree of fp16 tensor_tensor(max); max ~log2 live tiles
                stack = []  # (level, tile)
                for k in range(DA // 2):
                    t = pool.tile([128, width], bf16, tag=tag, name=f"{tag}_{k}")
                    nc.vector._custom_dve(
                        PAIRMAX,
                        out=t[:],
                        in0=in_of(2 * k),
                        in1=in_of(2 * k + 1),
                        s0=s_of(2 * k),
                        s1=s_of(2 * k + 1),
                    )
                    lvl = 0
                    while stack and stack[-1][0] == lvl:
                        _, prev = stack.pop()
                        nc.vector.tensor_tensor(prev[:], prev[:], t[:], Alu.max)
                        t = prev
                        lvl += 1
                    stack.append((lvl, t))
                while len(stack) > 1:
                    _, a = stack.pop()
                    _, b = stack.pop()
                    nc.vector.tensor_tensor(b[:], b[:], a[:], Alu.max)
                    stack.append((99, b))
                return stack[0][1]

            uET_ps = [psC.tile([128, US], f32, tag="u", name=f"uET{k}") for k in range(2)]
            for j in range(US):
                ab = apool.tile([DE, N], bf16, tag="uab")
                nc.scalar.activation(
                    out=ab[:],
                    in_=Acs[:],
                    func=AFT.Abs,
                    bias=McsN[:, j : j + 1],
                    scale=1.0,
                )
                for ch in range(2):
                    nc.tensor.matmul(
                        uET_ps[ch][:, j : j + 1],
                        ab[:, ch * 128 : (ch + 1) * 128],
                        ones_e[:],
                        start=True,
                        stop=True,
                    )
            ustack = []
            for k in range(DA // 2):
                t = pmpool.tile([128, 2 * US], bf16, tag="upmS", name=f"upmS_{k}")
                for ch in range(2):
                    nc.vector._custom_dve(
                        PAIRMAX,
                        out=t[:, ch * US : (ch + 1) * US],
                        in0=mrep[:, 2 * k * US : (2 * k + 1) * US],
                        in1=mrep[:, (2 * k + 1) * US : (2 * k + 2) * US],
                        s0=ga_s[ch][:, 2 * k : 2 * k + 1],
                        s1=ga_s[ch][:, 2 * k + 1 : 2 * k + 2],
                    )
                lvl = 0
                while ustack and ustack[-1][0] == lvl:
                    _, prev = ustack.pop()
                    nc.vector.tensor_tensor(prev[:], prev[:], t[:], Alu.max)
                    t = prev
                    lvl += 1
                ustack.append((lvl, t))
            while len(ustack) > 1:
                _, a = ustack.pop()
                _, b = ustack.pop()
                nc.vector.tensor_tensor(b[:], b[:], a[:], Alu.max)
                ustack.append((99, b))
            uacc_all = ustack[0][1]
            uni_p = []
            for ch in range(2):
                uacc = uacc_all[:, ch * US : (ch + 1) * US]
                uc1 = wpool.tile([128, US], f32, tag=f"uc1{ch}", name=f"uc1{ch}")
                nc.vector.tensor_scalar(
                    out=uc1[:], in0=uacc[:], scalar1=1.0, scalar2=None, op0=Alu.add
                )
                ur_a = wpool.tile([128, US], f32, tag=f"ura{ch}", name=f"ura{ch}")
                ursc = wpool.tile([128, US], f32, tag=f"ursc{ch}", name=f"ursc{ch}")
                nc.vector.reciprocal_approx_accurate(out=ur_a[:], in_=uc1[:], scratch=ursc[:])
                ux = wpool.tile([128, US], f32, tag=f"ux{ch}", name=f"ux{ch}")
                nc.vector.tensor_scalar(
                    out=ux[:],
                    in0=ur_a[:],
                    scalar1=-CA,
                    scalar2=CE + CA,
                    op0=Alu.mult,
                    op1=Alu.add,
                )
                ue1 = wpool.tile([128, US], f32, tag=f"ue1{ch}", name=f"ue1{ch}")
                nc.scalar.activation(
                    out=ue1[:], in_=uET_ps[ch][:], func=AFT.Copy, bias=1.0, scale=1.0
                )
                ur_e = wpool.tile([128, US], f32, tag=f"ure{ch}", name=f"ure{ch}")
                ursc2 = wpool.tile([128, US], f32, tag=f"ursc2{ch}", name=f"ursc2{ch}")
                nc.vector.reciprocal_approx_accurate(out=ur_e[:], in_=ue1[:], scratch=ursc2[:])
                ucomb = wpool.tile([128, US], f32, tag=f"ucomb{ch}", name=f"ucomb{ch}")
                nc.vector.scalar_tensor_tensor(
                    out=ucomb[:],
                    in0=ur_e[:],
                    scalar=-CE,
                    in1=ux[:],
                    op0=Alu.mult,
                    op1=Alu.add,
                )
                up = wpool.tile([128, 1], f32, tag=f"up{ch}", name=f"up{ch}")
                nc.vector.tensor_reduce(out=up[:], in_=ucomb[:], axis=X, op=Alu.add)
                uni_p.append(up)

            cin_u = dpool.tile([1, N], f32)
            nc.sync.dma_start(cin_u[:, 0:128], uni_p[0][:])
            nc.sync.dma_start(cin_u[:, 128:256], uni_p[1][:])
            cout_u = dpool.tile([NC, N], f32)
            nc.gpsimd.collective_compute(
                "AllGather",
                Alu.bypass,
                replica_groups=[list(range(NC))],
                ins=[cin_u.opt()],
                outs=[cout_u.opt()],
            )
            uni_cat = wpool.tile([1, NC * N], f32)
            nc.sync.dma_start(uni_cat[:], cout_u[:])
            uni_r = wpool.tile([1, N], f32)
            nc.vector.tensor_reduce(
                out=uni_r[:],
                in_=uni_cat.rearrange("p (c n) -> p n c", c=NC),
                axis=X,
                op=Alu.add,
            )

            # ---- ELMD nov via min identity: |a-b| = a+b-2*min(a,b) ------
            # ET[jt][j, i] = sum_d min(A_id, B_jd); SAB[jt][j, i] = 1+SA_i+SB_j
            # row/col sums SA [1,N], SB [1,MS] via PE ones-matmuls
            ET_ps = [
                psA.tile([128, N], f32, tag="big", name=f"ET{k}") for k in range(NJT)
            ]
            for i in range(N):
                ab = apool.tile([DE, MS], bf16, tag="ab")
                nc.scalar.activation(
                    out=ab[:],
                    in_=Bcs[:],
                    func=AFT.Abs,
                    bias=AcsN[:, i : i + 1],
                    scale=1.0,
                )
                for jt in range(NJT):
                    nc.tensor.matmul(
                        ET_ps[jt][:, i : i + 1],
                        ab[:, jt * 128 : (jt + 1) * 128],
                        ones_e[:],
                        start=True,
                        stop=True,
                    )

            # ---- AMD nov + combine per j-tile -> comb[jt] [128j, 256i] ----
            # all 4 j-tiles share one wide tile per pair so the merge tree
            # runs 4x-wide ops (fewer dispatches)
            stack = []
            for k in range(DA // 2):
                t = pmpool.tile([128, NJT * N], bf16, tag="pmS", name=f"pmS_{k}")
                for jt in range(NJT):
                    nc.vector._custom_dve(
                        PAIRMAX,
                        out=t[:, jt * N : (jt + 1) * N],
                        in0=arep[:, 2 * k * N : (2 * k + 1) * N],
                        in1=arep[:, (2 * k + 1) * N : (2 * k + 2) * N],
                        s0=ra_f[jt][:, 2 * k : 2 * k + 1],
                        s1=ra_f[jt][:, 2 * k + 1 : 2 * k + 2],
                    )
                lvl = 0
                while stack and stack[-1][0] == lvl:
                    _, prev = stack.pop()
                    nc.vector.tensor_tensor(prev[:], prev[:], t[:], Alu.max)
                    t = prev
                    lvl += 1
                stack.append((lvl, t))
            while len(stack) > 1:
                _, a = stack.pop()
                _, b = stack.pop()
                nc.vector.tensor_tensor(b[:], b[:], a[:], Alu.max)
                stack.append((99, b))
            acc_all = stack[0][1]
            accs = [acc_all[:, jt * N : (jt + 1) * N] for jt in range(NJT)]
            # combine per i-column half so the first half's tail work
            # (min tree, transpose, reduce) overlaps the second half's ELMD
            nov_p = [None, None]
            for h in range(2):
                sl = slice(h * 128, (h + 1) * 128)
                comb = []
                for jt in range(NJT):
                    c1f = accpool.tile([128, 128], f32, tag="c1f")
                    nc.scalar.activation(
                        out=c1f[:], in_=accs[jt][:, sl], func=AFT.Copy, bias=1.0, scale=1.0
                    )
                    r_a = accpool.tile([128, 128], f32, tag="r_a")
                    rsc = accpool.tile([128, 128], f32, tag="rsc")
                    nc.vector.reciprocal_approx_accurate(out=r_a[:], in_=c1f[:], scratch=rsc[:])
                    x = accpool.tile([128, 128], f32, tag="x")
                    nc.scalar.activation(
                        out=x[:], in_=r_a[:], func=AFT.Copy, bias=CE + CA, scale=-CA
                    )
                    e1 = accpool.tile([128, 128], f32, tag="e1")
                    nc.scalar.activation(
                        out=e1[:], in_=ET_ps[jt][:, sl], func=AFT.Copy, bias=1.0, scale=1.0
                    )
                    r_e = accpool.tile([128, 128], f32, tag="r_e")
                    rsc2 = accpool.tile([128, 128], f32, tag="rsc2")
                    nc.vector.reciprocal_approx_accurate(out=r_e[:], in_=e1[:], scratch=rsc2[:])
                    cb = combpool.tile([128, 128], f32, tag=f"comb{jt}", name=f"comb{h}_{jt}")
                    nc.vector.scalar_tensor_tensor(
                        out=cb[:],
                        in0=r_e[:],
                        scalar=-CE,
                        in1=x[:],
                        op0=Alu.mult,
                        op1=Alu.add,
                    )
                    comb.append(cb)
                nc.vector.tensor_tensor(comb[0][:], comb[0][:], comb[1][:], Alu.min)
                nc.vector.tensor_tensor(comb[2][:], comb[2][:], comb[3][:], Alu.min)
                nc.vector.tensor_tensor(comb[0][:], comb[0][:], comb[2][:], Alu.min)
                tr = psB.tile([128, 128], f32, tag="tr", name=f"tr{h}")
                nc.tensor.transpose(tr[:], comb[0][:], eye_s[:])
                np_t = combpool.tile([128, 1], f32, tag=f"novp{h}", name=f"novp{h}")
                nc.vector.tensor_reduce(out=np_t[:], in_=tr[:], axis=X, op=Alu.min)
                nov_p[h] = np_t

            # ---- uni (gen x gen, my 32 j columns): [128i, 32j] layout -----
            # ---------------- nov allgather (tail) --------------------------
            cin = dpool.tile([1, N], f32)
            nc.sync.dma_start(cin[:, 0:128], nov_p[0][:])
            nc.sync.dma_start(cin[:, 128:256], nov_p[1][:])
            cout = dpool.tile([NC, N], f32)
            nc.gpsimd.collective_compute(
                "AllGather",
                Alu.bypass,
                replica_groups=[list(range(NC))],
                ins=[cin.opt()],
                outs=[cout.opt()],
            )
            nov_cat = wpool.tile([1, NC * N], f32)
            nc.sync.dma_start(nov_cat[:], cout[:])
            nov_r = wpool.tile([1, N], f32)
            nc.vector.tensor_reduce(
                out=nov_r[:],
                in_=nov_cat.rearrange("p (c n) -> p n c", c=NC),
                axis=X,
                op=Alu.min,
            )

            # ---------------- final ----------------------------------------
            stab = wpool.tile([1, N], f32)
            nc.vector.tensor_scalar(
                out=stab[:],
                in0=eah_s[:],
                scalar1=-1.0 / EAH_SCALE,
                scalar2=1.0,
                op0=Alu.mult,
                op1=Alu.add,
            )
            nc.vector.tensor_scalar(
                out=stab[:],
                in0=stab[:],
                scalar1=0.0,
                scalar2=1.0,
                op0=Alu.max,
                op1=Alu.min,
            )
            s1 = wpool.tile([1, N], f32)
            nc.vector.tensor_tensor(s1[:], stab[:], nov_r[:], Alu.mult)
            s2 = wpool.tile([1, N], f32)
            nc.vector.tensor_scalar(
                out=s2[:],
                in0=uni_r[:],
                scalar1=1.0 / (N - 1),
                scalar2=None,
                op0=Alu.mult,
            )
            sc = wpool.tile([1, N], f32)
            nc.vector.tensor_tensor(sc[:], s1[:], s2[:], Alu.mult)
            nc.sync.dma_start(out[:], sc[:])
            nc.sync.dma_start(dbg_nov[:], nov_r[:])
            nc.sync.dma_start(dbg_uni[:], uni_r[:])
            nc.sync.dma_start(dbg_stab[:], stab[:])

    nc.compile()
    return nc


def _get_nc():
    if "nc" not in _CACHE:
        _CACHE["nc"] = _build()
    return _CACHE["nc"]


def _in_maps(inputs):
    ge = np.ascontiguousarray(inputs["gen_embs_elmd"], dtype=np.float32)
    ga = np.ascontiguousarray(inputs["gen_embs_amd"], dtype=np.float32)
    re = np.ascontiguousarray(inputs["ref_embs_elmd"], dtype=np.float32)
    ra = np.ascontiguousarray(inputs["ref_embs_amd"], dtype=np.float32)
    eah = np.ascontiguousarray(inputs["e_above_hull"], dtype=np.float32).reshape(1, N)
    tri = np.triu(np.ones((DE, DE), dtype=np.float32))
    eye = np.eye(128, dtype=np.float32)
    maps = []
    for c in range(NC):
        maps.append(
            {
                "ge": ge,
                "ga": ga,
                "re": np.ascontiguousarray(re[c * MS : (c + 1) * MS]),
                "ra": np.ascontiguousarray(ra[c * MS : (c + 1) * MS]),
                "mge": np.ascontiguousarray(ge[c * US : (c + 1) * US]),
                "mga": np.ascontiguousarray(ga[c * US : (c + 1) * US]),
                "eah": eah,
                "tri": tri,
                "eye": eye,
            }
        )
    return maps


def kernel(**inputs) -> np.ndarray:
    from concourse.bass_utils import run_bass_kernel_spmd

    nc = _get_nc()
    res = run_bass_kernel_spmd(nc, _in_maps(inputs), list(range(NC)))
    return np.asarray(res.results[0]["scores"], dtype=np.float32).reshape(N)
